# revision 6
# baseline (speedup 1.0000x reference)
"""ETC transient-global self-attention on 8 TRN2 NeuronCores.

Sharding: sequence-parallel. Core c handles example b = c//4, token rows
[1024*(c%4), 1024*(c%4+1)). Each core computes its q/k/v projections (k/v
with a 1-block halo), the per-example global (side) aggregates from the
full example, local+global attention, and the output projection for its
token rows. No cross-core communication; the host stacks the 8 row-slices.

Shapes (hardcoded from the problem spec):
  x  [2, 4096, 1024], Wq/Wk/Wv [1024, 16, 64], Wo [16, 64, 1024]
  block_len 128, 32 blocks, TOKENS_PER_BLOCK 16 -> G = 256 side tokens.

On-device layout notes:
  - everything runs in bf16 on the PE (f32 accumulate in PSUM).
  - attention logits are computed TRANSPOSED ([keys, q]) so that exp() is
    orientation-agnostic and PV contracts keys on the partition dim with no
    probs transpose. Softmax denominators come from a ones-column appended
    to v (PSUM row 64); the reference's extra-logit softmax needs no
    max-subtraction since logits are O(1) and masked entries multiply to 0.
"""

import numpy as np
import ml_dtypes

B, L, D, H, DH = 2, 4096, 1024, 16, 64
BL = 128                 # block length
NBLK = 32                # total blocks
G = 256                  # side (global) tokens
TPB = 16                 # tokens per side block
N_CORES = 8
NB = 8                   # blocks per core
TOK = NB * BL            # 1024 q tokens per core
KV = TOK + 2 * BL        # 1280 kv tokens (1-block halo each side)
BF16 = ml_dtypes.bfloat16

_PROG = None


def _build_program():
    import concourse.bass as bass
    import concourse.mybir as mybir
    import concourse.tile as tile
    from concourse import bacc
    from concourse.masks import make_identity

    dt = mybir.dt
    f32, bf16 = dt.float32, dt.bfloat16

    nc = bacc.Bacc("TRN2", target_bir_lowering=False, debug=False,
                   num_devices=N_CORES)

    xb_d = nc.declare_dram_parameter("xb", [L, D], bf16, isOutput=False)
    xkv_d = nc.declare_dram_parameter("xkv", [KV, D], bf16, isOutput=False)
    wq_d = nc.declare_dram_parameter("wq", [D, D], bf16, isOutput=False)
    wk_d = nc.declare_dram_parameter("wk", [D, D], bf16, isOutput=False)
    wv_d = nc.declare_dram_parameter("wv", [D, D], bf16, isOutput=False)
    wo_d = nc.declare_dram_parameter("wo", [D, D], bf16, isOutput=False)
    msk_d = nc.declare_dram_parameter("maskT", [BL, NB, 3, BL], bf16,
                                      isOutput=False)
    oh_d = nc.declare_dram_parameter("onehot", [BL, 16, BL], bf16,
                                     isOutput=False)
    out_d = nc.declare_dram_parameter("out", [TOK, D], f32, isOutput=True)

    Exp = mybir.ActivationFunctionType.Exp
    Cpy = mybir.ActivationFunctionType.Copy

    with tile.TileContext(nc) as tc:
        with (
            tc.tile_pool(name="per", bufs=1) as per,
            tc.tile_pool(name="strm", bufs=3) as strm,
            tc.tile_pool(name="att", bufs=2) as att,
            tc.tile_pool(name="osb", bufs=1) as osb,
        ):
            # ---- persistent SBUF tiles ----
            wk_sb = per.tile([128, 8, D], bf16, tag="wk")
            wv_sb = per.tile([128, 8, D], bf16, tag="wv")
            wo_sb = per.tile([128, 8, D], bf16, tag="wo")
            wq_sb = per.tile([128, 8, D], bf16, tag="wq_yT")   # dies -> yTf
            oh_sb = per.tile([128, 16, BL], bf16, tag="oh")
            msk_sb = per.tile([128, NB, 3, BL], bf16, tag="msk")
            ident = per.tile([128, 128], bf16, tag="ident")
            ones64 = per.tile([128, 64], bf16, tag="ones64")
            gnat = per.tile([128, 2, D], bf16, tag="gnat")
            gT = per.tile([128, 8, G], bf16, tag="gT")
            skT = per.tile([128, 8, G], bf16, tag="skT")
            svaug = per.tile([128, 2, H, DH + 1], bf16, tag="svaug")
            qT = per.tile([128, 8, TOK], bf16, tag="qT")
            kT = per.tile([128, 8, KV], bf16, tag="kT")
            vaug = per.tile([128, 10, H, DH + 1], bf16, tag="vaug")
            xT = per.tile([128, 8, KV], bf16, tag="xT_st")     # dies -> stage_o

            nc.sync.dma_start(out=wk_sb,
                              in_=wk_d.ap().rearrange("(c p) d -> p c d", p=128))
            nc.sync.dma_start(out=wv_sb,
                              in_=wv_d.ap().rearrange("(c p) d -> p c d", p=128))
            nc.sync.dma_start(out=wq_sb,
                              in_=wq_d.ap().rearrange("(c p) d -> p c d", p=128))
            nc.sync.dma_start(out=wo_sb,
                              in_=wo_d.ap().rearrange("(c p) d -> p c d", p=128))
            nc.sync.dma_start(out=oh_sb, in_=oh_d.ap())
            nc.sync.dma_start(out=msk_sb, in_=msk_d.ap())
            make_identity(nc, ident)
            nc.vector.memset(ones64, 1.0)

            with tc.tile_pool(name="psg", bufs=4, space="PSUM") as psg, \
                 tc.tile_pool(name="pst", bufs=2, space="PSUM") as pst, \
                 tc.tile_pool(name="psp", bufs=2, space="PSUM") as psp:

                # ---- phase B: side aggregation g (sum x over 16-token groups)
                gps = [psg.tile([128, 512], f32, tag="g", name=f"gps{i}")
                       for i in range(4)]
                for tc_i in range(32):
                    xt = strm.tile([128, D], bf16, tag="xs")
                    nc.sync.dma_start(out=xt, in_=xb_d[tc_i * 128:(tc_i + 1) * 128, :])
                    gt_i, tcl = tc_i // 16, tc_i % 16
                    for j in range(2):
                        nc.tensor.matmul(
                            gps[2 * gt_i + j],
                            oh_sb[:, tcl, :],
                            xt[:, 512 * j:512 * (j + 1)],
                            start=(tcl == 0), stop=(tcl == 15),
                        )
                    # phase C part 1: transposes of kv-range x chunks
                    # kv token range = [S0-128, S0+1152) = xb rows given by host
                    # via the separate xkv input; streamed below.
                for gt_i in range(2):
                    for j in range(2):
                        nc.vector.tensor_copy(
                            gnat[:, gt_i, 512 * j:512 * (j + 1)],
                            gps[2 * gt_i + j])

                # ---- phase C: build xT (kv range) and gT
                for t in range(10):
                    xt = strm.tile([128, D], bf16, tag="xs")
                    nc.sync.dma_start(out=xt, in_=xkv_d[t * 128:(t + 1) * 128, :])
                    for dc in range(8):
                        pt = pst.tile([128, 128], bf16, tag="tp")
                        nc.tensor.transpose(pt, xt[:, dc * 128:(dc + 1) * 128], ident)
                        nc.scalar.copy(xT[:, dc, t * 128:(t + 1) * 128], pt)
                for gt_i in range(2):
                    for dc in range(8):
                        pt = pst.tile([128, 128], bf16, tag="tp")
                        nc.tensor.transpose(
                            pt, gnat[:, gt_i, dc * 128:(dc + 1) * 128], ident)
                        nc.scalar.copy(gT[:, dc, gt_i * 128:(gt_i + 1) * 128], pt)

                # ---- phase D: projections ----
                # kT [2-head chunk oc, kv tokens]
                for oc in range(8):
                    for ts_, te in ((0, 512), (512, 1024), (1024, 1280)):
                        pp = psp.tile([128, 512], f32, tag="pj")
                        for dc in range(8):
                            nc.tensor.matmul(
                                pp[:, :te - ts_],
                                wk_sb[:, dc, oc * 128:(oc + 1) * 128],
                                xT[:, dc, ts_:te],
                                start=(dc == 0), stop=(dc == 7))
                        nc.vector.tensor_copy(kT[:, oc, ts_:te], pp[:, :te - ts_])
                # qT (q tokens = xT kv-rows 128..1152), Wq pre-scaled by 1/8
                for oc in range(8):
                    for tch in range(2):
                        ts_ = 128 + tch * 512
                        pp = psp.tile([128, 512], f32, tag="pj")
                        for dc in range(8):
                            nc.tensor.matmul(
                                pp,
                                wq_sb[:, dc, oc * 128:(oc + 1) * 128],
                                xT[:, dc, ts_:ts_ + 512],
                                start=(dc == 0), stop=(dc == 7))
                        nc.vector.tensor_copy(
                            qT[:, oc, tch * 512:(tch + 1) * 512], pp)
                # side kT
                for oc in range(8):
                    pp = psp.tile([128, 512], f32, tag="pj")
                    for dc in range(8):
                        nc.tensor.matmul(
                            pp[:, :G],
                            wk_sb[:, dc, oc * 128:(oc + 1) * 128],
                            gT[:, dc, :],
                            start=(dc == 0), stop=(dc == 7))
                    nc.vector.tensor_copy(skT[:, oc, :], pp[:, :G])
                # v natural (augmented with ones column per head)
                for t in range(10):
                    for j in range(2):
                        pp = psp.tile([128, 512], f32, tag="pj")
                        for dc in range(8):
                            nc.tensor.matmul(
                                pp,
                                xT[:, dc, t * 128:(t + 1) * 128],
                                wv_sb[:, dc, 512 * j:512 * (j + 1)],
                                start=(dc == 0), stop=(dc == 7))
                        nc.scalar.copy(
                            vaug[:, t, 8 * j:8 * (j + 1), 0:DH],
                            pp.rearrange("p (h d) -> p h d", h=8))
                    nc.vector.memset(vaug[:, t, :, DH:DH + 1], 1.0)
                # side v (augmented)
                for gt_i in range(2):
                    for j in range(2):
                        pp = psp.tile([128, 512], f32, tag="pj")
                        for dc in range(8):
                            nc.tensor.matmul(
                                pp,
                                gT[:, dc, gt_i * 128:(gt_i + 1) * 128],
                                wv_sb[:, dc, 512 * j:512 * (j + 1)],
                                start=(dc == 0), stop=(dc == 7))
                        nc.scalar.copy(
                            svaug[:, gt_i, 8 * j:8 * (j + 1), 0:DH],
                            pp.rearrange("p (h d) -> p h d", h=8))
                    nc.vector.memset(svaug[:, gt_i, :, DH:DH + 1], 1.0)

            # ---- phase E: attention ----
            yTf = per.tile([128, 8, TOK], bf16, tag="wq_yT")
            stage_o = per.tile([64, 8, TOK], bf16, tag="xT_st")
            with tc.tile_pool(name="plg", bufs=2, space="PSUM") as plg, \
                 tc.tile_pool(name="psg2", bufs=2, space="PSUM") as psg2, \
                 tc.tile_pool(name="pyt", bufs=2, space="PSUM") as pyt, \
                 tc.tile_pool(name="pbc", bufs=2, space="PSUM") as pbc:
                for n in range(NB):
                    for h in range(H):
                        oc, r0 = h // 2, 64 * (h % 2)
                        qs = qT[r0:r0 + 64, oc, n * 128:(n + 1) * 128]
                        # QK local (3 chunks) and side (2 chunks), transposed
                        lg = plg.tile([128, 3, 128], f32, tag="lg")
                        for c in range(3):
                            nc.tensor.matmul(
                                lg[:, c, :],
                                kT[r0:r0 + 64, oc, (n + c) * 128:(n + c + 1) * 128],
                                qs, start=True, stop=True)
                        sg = psg2.tile([128, 2, 128], f32, tag="sg")
                        for gt_i in range(2):
                            nc.tensor.matmul(
                                sg[:, gt_i, :],
                                skT[r0:r0 + 64, oc, gt_i * 128:(gt_i + 1) * 128],
                                qs, start=True, stop=True)
                        # exp (+ mask via multiply)
                        ul = att.tile([128, 3, 128], bf16, tag="ul")
                        nc.scalar.activation(ul, lg, Exp)
                        nc.vector.tensor_mul(ul, ul, msk_sb[:, n, :, :])
                        us = att.tile([128, 2, 128], bf16, tag="us")
                        nc.scalar.activation(us, sg, Exp)
                        # PV (+ denominator row 64)
                        yt = pyt.tile([65, 128], f32, tag="yt")
                        for c in range(3):
                            nc.tensor.matmul(
                                yt, vaug[:, n + c, h, :], ul[:, c, :],
                                start=(c == 0), stop=False)
                        for gt_i in range(2):
                            nc.tensor.matmul(
                                yt, svaug[:, gt_i, h, :], us[:, gt_i, :],
                                start=False, stop=(gt_i == 1))
                        # normalize: recip(denom+1) at partition 64, broadcast
                        rc = att.tile([128, 128], bf16, tag="rc")
                        nc.scalar.activation(rc[64:65, :], yt[64:65, :], Cpy,
                                             bias=1.0)
                        with nc.allow_low_precision(reason="bf16 softmax recip"):
                            nc.vector.reciprocal(rc[64:65, :], rc[64:65, :])
                        bc = pbc.tile([64, 128], f32, tag="bc")
                        nc.tensor.matmul(bc, ones64[64:65, :], rc[64:65, :],
                                         start=True, stop=True)
                        ysb = att.tile([64, 128], bf16, tag="ysb")
                        nc.scalar.copy(ysb, yt[0:64, :])
                        dst = (yTf[0:64, oc, n * 128:(n + 1) * 128]
                               if h % 2 == 0 else
                               stage_o[0:64, oc, n * 128:(n + 1) * 128])
                        nc.vector.tensor_mul(dst, ysb, bc)
                # shift odd heads up to partitions 64..127 (one big DMA)
                nc.sync.dma_start(out=yTf[64:128, :, :], in_=stage_o[0:64, :, :])

            # ---- phase F: output projection ----
            with tc.tile_pool(name="pso", bufs=2, space="PSUM") as pso:
                for tt in range(8):
                    ot = osb.tile([128, D], f32, tag="ot")
                    for j in range(2):
                        pp = pso.tile([128, 512], f32, tag="po")
                        for oc in range(8):
                            nc.tensor.matmul(
                                pp,
                                yTf[:, oc, tt * 128:(tt + 1) * 128],
                                wo_sb[:, oc, 512 * j:512 * (j + 1)],
                                start=(oc == 0), stop=(oc == 7))
                        nc.vector.tensor_copy(ot[:, 512 * j:512 * (j + 1)], pp)
                    nc.sync.dma_start(out=out_d[tt * 128:(tt + 1) * 128, :],
                                      in_=ot)

    nc.compile()
    return nc


def _host_inputs(x, Wq, Wk, Wv, Wo):
    """Build the 8 per-core input maps (all numpy, bf16 where device expects)."""
    xbf = x.astype(BF16)
    wq = (Wq.reshape(D, D).astype(np.float32) / np.sqrt(DH)).astype(BF16)
    wk = Wk.reshape(D, D).astype(BF16)
    wv = Wv.reshape(D, D).astype(BF16)
    wo = Wo.reshape(D, D).astype(BF16)

    # onehot2[t, s, c]: 1 where c == 8*s + t//16  (shift-s aggregation weights)
    t_ = np.arange(BL)
    oh = np.zeros((BL, 16, BL), np.float32)
    for s in range(16):
        oh[t_, s, 8 * s + t_ // 16] = 1.0
    oh = oh.astype(BF16)

    in_maps = []
    for c in range(N_CORES):
        b, s = c // 4, c % 4
        S0 = s * TOK
        blk0 = S0 // BL
        xkv = np.zeros((KV, D), BF16)
        a0 = S0 - BL
        lo, hi = max(a0, 0), min(a0 + KV, L)
        xkv[lo - a0:hi - a0] = xbf[b, lo:hi]
        # maskT[k, n, c, q]: local-window validity, transposed
        k_ = np.arange(BL)[:, None, None, None]
        n_ = np.arange(NB)[None, :, None, None]
        c_ = np.arange(3)[None, None, :, None]
        q_ = np.arange(BL)[None, None, None, :]
        rel = (c_ * BL + k_) - BL - q_
        kpos = (blk0 + n_ - 1) * BL + c_ * BL + k_
        valid = (np.abs(rel) <= BL - 1) & (kpos >= 0) & (kpos < L)
        in_maps.append({
            "xb": np.ascontiguousarray(xbf[b]),
            "xkv": xkv,
            "wq": wq, "wk": wk, "wv": wv, "wo": wo,
            "maskT": valid.astype(BF16),
            "onehot": oh,
        })
    return in_maps


def kernel(x, Wq, Wk, Wv, Wo):
    from concourse.bass_utils import run_bass_kernel_spmd

    global _PROG
    if _PROG is None:
        _PROG = _build_program()
    in_maps = _host_inputs(np.asarray(x, np.float32), np.asarray(Wq, np.float32),
                           np.asarray(Wk, np.float32), np.asarray(Wv, np.float32),
                           np.asarray(Wo, np.float32))
    res = run_bass_kernel_spmd(_PROG, in_maps, list(range(N_CORES)))
    out = np.empty((B, L, D), np.float32)
    for c in range(N_CORES):
        b, s = c // 4, c % 4
        out[b, s * TOK:(s + 1) * TOK] = res.results[c]["out"]
    return out


# revision 15
# speedup vs baseline: 13464.4228x; 13464.4228x over previous
"""ETC transient-global self-attention on 8 TRN2 NeuronCores.

Sharding: sequence-parallel. Core c handles example b = c//4, token rows
[1024*(c%4), 1024*(c%4+1)). Each core computes its q/k/v projections (k/v
with a 1-block halo), the per-example global (side) aggregates from the
full example, local+global attention, and the output projection for its
token rows. No cross-core communication; the host stacks the 8 row-slices.

Shapes (hardcoded from the problem spec):
  x  [2, 4096, 1024], Wq/Wk/Wv [1024, 16, 64], Wo [16, 64, 1024]
  block_len 128, 32 blocks, TOKENS_PER_BLOCK 16 -> G = 256 side tokens.

On-device layout notes:
  - everything runs in bf16 on the PE (f32 accumulate in PSUM).
  - attention logits are computed TRANSPOSED ([keys, q]) so that exp() is
    orientation-agnostic and PV contracts keys on the partition dim with no
    probs transpose. Softmax denominators come from a ones-column appended
    to v (PSUM row 64); the reference's extra-logit softmax needs no
    max-subtraction since logits are O(1) and masked entries multiply to 0.
"""

import numpy as np
import ml_dtypes

B, L, D, H, DH = 2, 4096, 1024, 16, 64
BL = 128                 # block length
NBLK = 32                # total blocks
G = 256                  # side (global) tokens
TPB = 16                 # tokens per side block
N_CORES = 8
NB = 8                   # blocks per core
TOK = NB * BL            # 1024 q tokens per core
KV = TOK + 2 * BL        # 1280 kv tokens (1-block halo each side)
BF16 = ml_dtypes.bfloat16

_PROG = None


def _build_program():
    import concourse.bass as bass
    import concourse.mybir as mybir
    import concourse.tile as tile
    from concourse import bacc
    from concourse.masks import make_identity

    dt = mybir.dt
    f32, bf16 = dt.float32, dt.bfloat16

    nc = bacc.Bacc("TRN2", target_bir_lowering=False, debug=False,
                   num_devices=N_CORES)

    xb_d = nc.declare_dram_parameter("xb", [L, D], bf16, isOutput=False)
    xkv_d = nc.declare_dram_parameter("xkv", [KV, D], bf16, isOutput=False)
    wq_d = nc.declare_dram_parameter("wq", [D, D], bf16, isOutput=False)
    wk_d = nc.declare_dram_parameter("wk", [D, D], bf16, isOutput=False)
    wv_d = nc.declare_dram_parameter("wv", [D, D], bf16, isOutput=False)
    wo_d = nc.declare_dram_parameter("wo", [D, D], bf16, isOutput=False)
    msk_d = nc.declare_dram_parameter("maskT", [BL, NB, 3, BL], bf16,
                                      isOutput=False)
    oh_d = nc.declare_dram_parameter("onehot", [BL, 16, BL], bf16,
                                     isOutput=False)
    out_d = nc.declare_dram_parameter("out", [TOK, D], f32, isOutput=True)

    Exp = mybir.ActivationFunctionType.Exp
    Cpy = mybir.ActivationFunctionType.Copy

    with tile.TileContext(nc) as tc:
        with (
            tc.tile_pool(name="per", bufs=1) as per,
            tc.tile_pool(name="strm", bufs=3) as strm,
            tc.tile_pool(name="att", bufs=2) as att,
            tc.tile_pool(name="osb", bufs=2) as osb,
        ):
            # ---- persistent SBUF tiles ----
            wk_sb = per.tile([128, 8, D], bf16, tag="wk")
            wv_sb = per.tile([128, 8, D], bf16, tag="wv")
            wo_sb = per.tile([128, 8, D], bf16, tag="wo")
            wq_sb = per.tile([128, 8, D], bf16, tag="wq_yT")   # dies -> yTf
            oh_sb = per.tile([128, 16, BL], bf16, tag="oh")
            msk_sb = per.tile([128, NB, 3, BL], bf16, tag="msk")
            ident = per.tile([128, 128], bf16, tag="ident")
            ones64 = per.tile([128, 64], bf16, tag="ones64")
            gnat = per.tile([128, 2, D], bf16, tag="gnat")
            gT = per.tile([128, 8, G], bf16, tag="gT")
            skT = per.tile([128, 8, G], bf16, tag="skT")
            svaug = per.tile([128, 2, H, DH + 1], bf16, tag="svaug")
            qT = per.tile([128, 8, TOK], bf16, tag="qT")
            kT = per.tile([128, 8, KV], bf16, tag="kT")
            vaug = per.tile([128, 10, H, DH + 1], bf16, tag="vaug")
            xT = per.tile([128, 8, KV], bf16, tag="xT_st")     # dies -> stage_o

            # DMA issue order matters for ramp-up: inputs feeding the first
            # compute (onehot + xb stream for the aggregation, xT transpose
            # loads) go first; weights arrive later, in first-use order.
            nc.sync.dma_start(out=oh_sb, in_=oh_d.ap())
            # xT built directly by 2-byte transpose DMAs (one per D-chunk)
            for dc in range(8):
                nc.sync.dma_start(out=xT[:, dc, :],
                                  in_=xkv_d[:, dc * 128:(dc + 1) * 128],
                                  transpose=True)
            nc.sync.dma_start(out=msk_sb, in_=msk_d.ap())
            nc.sync.dma_start(out=wv_sb,
                              in_=wv_d.ap().rearrange("(c p) d -> p c d", p=128))
            nc.sync.dma_start(out=wk_sb,
                              in_=wk_d.ap().rearrange("(c p) d -> p c d", p=128))
            nc.sync.dma_start(out=wq_sb,
                              in_=wq_d.ap().rearrange("(c p) d -> p c d", p=128))
            nc.sync.dma_start(out=wo_sb,
                              in_=wo_d.ap().rearrange("(c p) d -> p c d", p=128))
            make_identity(nc, ident)
            nc.vector.memset(ones64, 1.0)

            with tc.tile_pool(name="psg", bufs=4, space="PSUM") as psg, \
                 tc.tile_pool(name="pst", bufs=2, space="PSUM") as pst, \
                 tc.tile_pool(name="psp", bufs=2, space="PSUM") as psp:

                # ---- phase B: side aggregation g (sum x over 16-token groups)
                gps = [psg.tile([128, 512], f32, tag="g", name=f"gps{i}")
                       for i in range(4)]
                for tc_i in range(32):
                    xt = strm.tile([128, D], bf16, tag="xs")
                    nc.sync.dma_start(out=xt, in_=xb_d[tc_i * 128:(tc_i + 1) * 128, :])
                    gt_i, tcl = tc_i // 16, tc_i % 16
                    for j in range(2):
                        nc.tensor.matmul(
                            gps[2 * gt_i + j],
                            oh_sb[:, tcl, :],
                            xt[:, 512 * j:512 * (j + 1)],
                            start=(tcl == 0), stop=(tcl == 15),
                        )
                    # phase C part 1: transposes of kv-range x chunks
                    # kv token range = [S0-128, S0+1152) = xb rows given by host
                    # via the separate xkv input; streamed below.
                for gt_i in range(2):
                    for j in range(2):
                        nc.vector.tensor_copy(
                            gnat[:, gt_i, 512 * j:512 * (j + 1)],
                            gps[2 * gt_i + j])

                # ---- phase C: build gT (xT comes straight from transpose DMA)
                for gt_i in range(2):
                    for dc in range(8):
                        pt = pst.tile([128, 128], bf16, tag="tp")
                        nc.tensor.transpose(
                            pt, gnat[:, gt_i, dc * 128:(dc + 1) * 128], ident)
                        nc.scalar.copy(gT[:, dc, gt_i * 128:(gt_i + 1) * 128], pt)

                # ---- phase D: projections ----
                # kT [2-head chunk oc, kv tokens]
                for oc in range(8):
                    for ts_, te in ((0, 512), (512, 1024), (1024, 1280)):
                        pp = psp.tile([128, 512], f32, tag="pj")
                        for dc in range(8):
                            nc.tensor.matmul(
                                pp[:, :te - ts_],
                                wk_sb[:, dc, oc * 128:(oc + 1) * 128],
                                xT[:, dc, ts_:te],
                                start=(dc == 0), stop=(dc == 7))
                        nc.vector.tensor_copy(kT[:, oc, ts_:te], pp[:, :te - ts_])
                # qT (q tokens = xT kv-rows 128..1152), Wq pre-scaled by 1/8
                for oc in range(8):
                    for tch in range(2):
                        ts_ = 128 + tch * 512
                        pp = psp.tile([128, 512], f32, tag="pj")
                        for dc in range(8):
                            nc.tensor.matmul(
                                pp,
                                wq_sb[:, dc, oc * 128:(oc + 1) * 128],
                                xT[:, dc, ts_:ts_ + 512],
                                start=(dc == 0), stop=(dc == 7))
                        nc.vector.tensor_copy(
                            qT[:, oc, tch * 512:(tch + 1) * 512], pp)
                # side kT
                for oc in range(8):
                    pp = psp.tile([128, 512], f32, tag="pj")
                    for dc in range(8):
                        nc.tensor.matmul(
                            pp[:, :G],
                            wk_sb[:, dc, oc * 128:(oc + 1) * 128],
                            gT[:, dc, :],
                            start=(dc == 0), stop=(dc == 7))
                    nc.vector.tensor_copy(skT[:, oc, :], pp[:, :G])
                # v natural (augmented with ones column per head)
                for t in range(10):
                    for j in range(2):
                        pp = psp.tile([128, 512], f32, tag="pj")
                        for dc in range(8):
                            nc.tensor.matmul(
                                pp,
                                xT[:, dc, t * 128:(t + 1) * 128],
                                wv_sb[:, dc, 512 * j:512 * (j + 1)],
                                start=(dc == 0), stop=(dc == 7))
                        nc.scalar.copy(
                            vaug[:, t, 8 * j:8 * (j + 1), 0:DH],
                            pp.rearrange("p (h d) -> p h d", h=8))
                    nc.vector.memset(vaug[:, t, :, DH:DH + 1], 1.0)
                # side v (augmented)
                for gt_i in range(2):
                    for j in range(2):
                        pp = psp.tile([128, 512], f32, tag="pj")
                        for dc in range(8):
                            nc.tensor.matmul(
                                pp,
                                gT[:, dc, gt_i * 128:(gt_i + 1) * 128],
                                wv_sb[:, dc, 512 * j:512 * (j + 1)],
                                start=(dc == 0), stop=(dc == 7))
                        nc.scalar.copy(
                            svaug[:, gt_i, 8 * j:8 * (j + 1), 0:DH],
                            pp.rearrange("p (h d) -> p h d", h=8))
                    nc.vector.memset(svaug[:, gt_i, :, DH:DH + 1], 1.0)

            # ---- phase E: attention ----
            yTf = per.tile([128, 8, TOK], bf16, tag="wq_yT")
            stage_o = per.tile([64, 8, TOK], bf16, tag="xT_st")
            with tc.tile_pool(name="plg", bufs=3, space="PSUM") as plg, \
                 tc.tile_pool(name="psg2", bufs=2, space="PSUM") as psg2, \
                 tc.tile_pool(name="pyt", bufs=2, space="PSUM") as pyt, \
                 tc.tile_pool(name="pbc", bufs=1, space="PSUM") as pbc:
                for h in range(H):
                    oc, r0 = h // 2, 64 * (h % 2)
                    for nh in range(2):          # half = 4 blocks = 512 q
                        q4 = qT[r0:r0 + 64, oc, nh * 512:(nh + 1) * 512]
                        # side QK batched over the 4 blocks (N=512)
                        sgs = [psg2.tile([128, 512], f32, tag="sg",
                                         name=f"sg{h}_{nh}_{g}")
                               for g in range(2)]
                        us = att.tile([128, 2, 512], bf16, tag="us")
                        for g in range(2):
                            nc.tensor.matmul(
                                sgs[g],
                                skT[r0:r0 + 64, oc, g * 128:(g + 1) * 128],
                                q4, start=True, stop=True)
                            nc.scalar.activation(us[:, g, :], sgs[g], Exp)
                        # local QK + exp + mask per block
                        uls = []
                        for i in range(4):
                            n = nh * 4 + i
                            qs = qT[r0:r0 + 64, oc, n * 128:(n + 1) * 128]
                            lg = plg.tile([128, 3, 128], f32, tag="lg",
                                          name=f"lg{h}_{n}")
                            for c in range(3):
                                nc.tensor.matmul(
                                    lg[:, c, :],
                                    kT[r0:r0 + 64, oc,
                                       (n + c) * 128:(n + c + 1) * 128],
                                    qs, start=True, stop=True)
                            ul = att.tile([128, 3, 128], bf16, tag="ul", bufs=6,
                                          name=f"ul{h}_{n}")
                            nc.scalar.activation(ul, lg, Exp)
                            nc.vector.tensor_mul(ul, ul, msk_sb[:, n, :, :])
                            uls.append(ul)
                        # PV for 4 blocks into one [65, 512] psum
                        yt = pyt.tile([65, 512], f32, tag="yt")
                        for i in range(4):
                            n = nh * 4 + i
                            for c in range(3):
                                # start=True clears the whole PSUM tile, so
                                # only the very first matmul of the group may
                                # set it; per-element has_written handles the
                                # first write of each column slice.
                                nc.tensor.matmul(
                                    yt[:, i * 128:(i + 1) * 128],
                                    vaug[:, n + c, h, :], uls[i][:, c, :],
                                    start=(i == 0 and c == 0), stop=False)
                        for g in range(2):
                            nc.tensor.matmul(
                                yt, svaug[:, g, h, :], us[:, g, :],
                                start=False, stop=(g == 1))
                        # normalize: bcast (denom+1), lane-parallel recip, mul
                        rc = att.tile([128, 512], bf16, tag="rc", bufs=3)
                        nc.vector.tensor_scalar_add(rc[64:65, :], yt[64:65, :],
                                                    1.0)
                        bc = pbc.tile([64, 512], f32, tag="bc")
                        nc.tensor.matmul(bc, ones64[64:65, :], rc[64:65, :],
                                         start=True, stop=True)
                        rcb = att.tile([64, 512], bf16, tag="rcb", bufs=3)
                        with nc.allow_low_precision(reason="bf16 softmax recip"):
                            nc.vector.reciprocal(rcb, bc)
                        dst = (yTf[0:64, oc, nh * 512:(nh + 1) * 512]
                               if h % 2 == 0 else
                               stage_o[0:64, oc, nh * 512:(nh + 1) * 512])
                        nc.vector.tensor_mul(dst, yt[0:64, :], rcb)
                # shift odd heads up to partitions 64..127 (one big DMA)
                nc.sync.dma_start(out=yTf[64:128, :, :], in_=stage_o[0:64, :, :])

            # ---- phase F: output projection ----
            with tc.tile_pool(name="pso", bufs=2, space="PSUM") as pso:
                for tt in range(8):
                    ot = osb.tile([128, D], f32, tag="ot")
                    for j in range(2):
                        pp = pso.tile([128, 512], f32, tag="po")
                        for oc in range(8):
                            nc.tensor.matmul(
                                pp,
                                yTf[:, oc, tt * 128:(tt + 1) * 128],
                                wo_sb[:, oc, 512 * j:512 * (j + 1)],
                                start=(oc == 0), stop=(oc == 7))
                        nc.vector.tensor_copy(ot[:, 512 * j:512 * (j + 1)], pp)
                    nc.sync.dma_start(out=out_d[tt * 128:(tt + 1) * 128, :],
                                      in_=ot)

    nc.compile()
    return nc


def _host_inputs(x, Wq, Wk, Wv, Wo):
    """Build the 8 per-core input maps (all numpy, bf16 where device expects)."""
    xbf = x.astype(BF16)
    wq = (Wq.reshape(D, D).astype(np.float32) / np.sqrt(DH)).astype(BF16)
    wk = Wk.reshape(D, D).astype(BF16)
    wv = Wv.reshape(D, D).astype(BF16)
    wo = Wo.reshape(D, D).astype(BF16)

    # onehot2[t, s, c]: 1 where c == 8*s + t//16  (shift-s aggregation weights)
    t_ = np.arange(BL)
    oh = np.zeros((BL, 16, BL), np.float32)
    for s in range(16):
        oh[t_, s, 8 * s + t_ // 16] = 1.0
    oh = oh.astype(BF16)

    in_maps = []
    for c in range(N_CORES):
        b, s = c // 4, c % 4
        S0 = s * TOK
        blk0 = S0 // BL
        xkv = np.zeros((KV, D), BF16)
        a0 = S0 - BL
        lo, hi = max(a0, 0), min(a0 + KV, L)
        xkv[lo - a0:hi - a0] = xbf[b, lo:hi]
        # maskT[k, n, c, q]: local-window validity, transposed
        k_ = np.arange(BL)[:, None, None, None]
        n_ = np.arange(NB)[None, :, None, None]
        c_ = np.arange(3)[None, None, :, None]
        q_ = np.arange(BL)[None, None, None, :]
        rel = (c_ * BL + k_) - BL - q_
        kpos = (blk0 + n_ - 1) * BL + c_ * BL + k_
        valid = (np.abs(rel) <= BL - 1) & (kpos >= 0) & (kpos < L)
        in_maps.append({
            "xb": np.ascontiguousarray(xbf[b]),
            "xkv": xkv,
            "wq": wq, "wk": wk, "wv": wv, "wo": wo,
            "maskT": valid.astype(BF16),
            "onehot": oh,
        })
    return in_maps


_RUNNER = None


def _make_runner(nc):
    """Build the PJRT executable once; returns fn(in_maps) -> per-core outs.

    Mirrors concourse.bass2jax.run_bass_via_pjrt, but caches the jitted
    shard_map callable so repeat kernel() calls skip retrace/recompile.
    """
    import jax
    import numpy as _np
    from jax.sharding import Mesh, PartitionSpec
    from jax.experimental.shard_map import shard_map
    import concourse.mybir as mybir
    from concourse import bass2jax

    bass2jax.install_neuronx_cc_hook()
    partition_name = (nc.partition_id_tensor.name
                      if nc.partition_id_tensor else None)
    in_names, out_names, out_avals = [], [], []
    for alloc in nc.m.functions[0].allocations:
        if not isinstance(alloc, mybir.MemoryLocationSet):
            continue
        name = alloc.memorylocations[0].name
        if alloc.kind == "ExternalInput":
            if name != partition_name:
                in_names.append(name)
        elif alloc.kind == "ExternalOutput":
            out_avals.append(jax.core.ShapedArray(
                tuple(alloc.tensor_shape), mybir.dt.np(alloc.dtype)))
            out_names.append(name)
    n_params = len(in_names)
    all_names = in_names + out_names
    if partition_name is not None:
        all_names.append(partition_name)
    donate = tuple(range(n_params, n_params + len(out_names)))

    def _body(*args):
        operands = list(args)
        if partition_name is not None:
            operands.append(bass2jax.partition_id_tensor())
        return tuple(bass2jax._bass_exec_p.bind(
            *operands, out_avals=tuple(out_avals), in_names=tuple(all_names),
            out_names=tuple(out_names), lowering_input_output_aliases=(),
            sim_require_finite=True, sim_require_nnan=True, nc=nc))

    devices = jax.devices()[:N_CORES]
    mesh = Mesh(_np.asarray(devices), ("core",))
    specs = (PartitionSpec("core"),) * (n_params + len(out_names))
    sharded = jax.jit(
        shard_map(_body, mesh=mesh, in_specs=specs,
                  out_specs=(PartitionSpec("core"),) * len(out_names),
                  check_rep=False),
        donate_argnums=donate, keep_unused=True)

    def run(in_maps):
        concat_in = [
            _np.concatenate([_np.asarray(in_maps[c][k]) for c in range(N_CORES)],
                            axis=0)
            for k in in_names
        ]
        concat_zeros = [_np.zeros((N_CORES * a.shape[0], *a.shape[1:]), a.dtype)
                        for a in out_avals]
        outs = sharded(*concat_in, *concat_zeros)
        return [
            {k: _np.asarray(outs[i]).reshape(N_CORES, *out_avals[i].shape)[c]
             for i, k in enumerate(out_names)}
            for c in range(N_CORES)
        ]

    return run


def kernel(x, Wq, Wk, Wv, Wo):
    global _PROG, _RUNNER
    if _RUNNER is None:
        _PROG = _build_program()
        _RUNNER = _make_runner(_PROG)
    in_maps = _host_inputs(np.asarray(x, np.float32), np.asarray(Wq, np.float32),
                           np.asarray(Wk, np.float32), np.asarray(Wv, np.float32),
                           np.asarray(Wo, np.float32))
    results = _RUNNER(in_maps)
    out = np.empty((B, L, D), np.float32)
    for c in range(N_CORES):
        b, s = c // 4, c % 4
        out[b, s * TOK:(s + 1) * TOK] = results[c]["out"]
    return out


# revision 17
# speedup vs baseline: 14194.8747x; 1.0543x over previous
"""ETC transient-global self-attention on 8 TRN2 NeuronCores.

Sharding: sequence-parallel. Core c handles example b = c//4, token rows
[1024*(c%4), 1024*(c%4+1)). Each core computes its q/k/v projections (k/v
with a 1-block halo), the per-example global (side) aggregates from the
full example, local+global attention, and the output projection for its
token rows. No cross-core communication; the host stacks the 8 row-slices.

Shapes (hardcoded from the problem spec):
  x  [2, 4096, 1024], Wq/Wk/Wv [1024, 16, 64], Wo [16, 64, 1024]
  block_len 128, 32 blocks, TOKENS_PER_BLOCK 16 -> G = 256 side tokens.

On-device layout notes:
  - everything runs in bf16 on the PE (f32 accumulate in PSUM).
  - attention logits are computed TRANSPOSED ([keys, q]) so that exp() is
    orientation-agnostic and PV contracts keys on the partition dim with no
    probs transpose. Softmax denominators come from a ones-column appended
    to v (PSUM row 64); the reference's extra-logit softmax needs no
    max-subtraction since logits are O(1) and masked entries multiply to 0.
"""

import numpy as np
import ml_dtypes

B, L, D, H, DH = 2, 4096, 1024, 16, 64
BL = 128                 # block length
NBLK = 32                # total blocks
G = 256                  # side (global) tokens
TPB = 16                 # tokens per side block
N_CORES = 8
NB = 8                   # blocks per core
TOK = NB * BL            # 1024 q tokens per core
KV = TOK + 2 * BL        # 1280 kv tokens (1-block halo each side)
BF16 = ml_dtypes.bfloat16

_PROG = None


def _build_program():
    import concourse.bass as bass
    import concourse.mybir as mybir
    import concourse.tile as tile
    from concourse import bacc
    from concourse.masks import make_identity

    dt = mybir.dt
    f32, bf16 = dt.float32, dt.bfloat16

    nc = bacc.Bacc("TRN2", target_bir_lowering=False, debug=False,
                   num_devices=N_CORES)

    xkv_d = nc.declare_dram_parameter("xkv", [KV, D], bf16, isOutput=False)
    g_d = nc.declare_dram_parameter("g", [G, D], bf16, isOutput=False)
    wq_d = nc.declare_dram_parameter("wq", [D, D], bf16, isOutput=False)
    wk_d = nc.declare_dram_parameter("wk", [D, D], bf16, isOutput=False)
    wv_d = nc.declare_dram_parameter("wv", [D, D], bf16, isOutput=False)
    wo_d = nc.declare_dram_parameter("wo", [D, D], bf16, isOutput=False)
    msk_d = nc.declare_dram_parameter("maskT", [BL, NB, 3, BL], bf16,
                                      isOutput=False)
    out_d = nc.declare_dram_parameter("out", [TOK, D], f32, isOutput=True)

    Exp = mybir.ActivationFunctionType.Exp
    Cpy = mybir.ActivationFunctionType.Copy

    with tile.TileContext(nc) as tc:
        with (
            tc.tile_pool(name="per", bufs=1) as per,
            tc.tile_pool(name="strm", bufs=3) as strm,
            tc.tile_pool(name="att", bufs=2) as att,
            tc.tile_pool(name="osb", bufs=2) as osb,
        ):
            # ---- persistent SBUF tiles ----
            wk_sb = per.tile([128, 8, D], bf16, tag="wk")
            wv_sb = per.tile([128, 8, D], bf16, tag="wv")
            wo_sb = per.tile([128, 8, D], bf16, tag="wo")
            wq_sb = per.tile([128, 8, D], bf16, tag="wq_yT")   # dies -> yTf
            msk_sb = per.tile([128, NB, 3, BL], bf16, tag="msk")
            ident = per.tile([128, 128], bf16, tag="ident")
            ones64 = per.tile([128, 64], bf16, tag="ones64")
            gnat = per.tile([128, 2, D], bf16, tag="gnat")
            gT = per.tile([128, 8, G], bf16, tag="gT")
            skT = per.tile([128, 8, G], bf16, tag="skT")
            svaug = per.tile([128, 2, H, DH + 1], bf16, tag="svaug")
            qT = per.tile([128, 8, TOK], bf16, tag="qT")
            kT = per.tile([128, 8, KV], bf16, tag="kT")
            vaug = per.tile([128, 10, H, DH + 1], bf16, tag="vaug")
            xT = per.tile([128, 8, KV], bf16, tag="xT_st")     # dies -> stage_o

            # DMA issue order matters for ramp-up: inputs feeding the first
            # compute (g + xT transpose loads) go first; weights arrive
            # later, in first-use order.
            nc.sync.dma_start(out=gnat,
                              in_=g_d.ap().rearrange("(t p) d -> p t d", p=128))
            # xT built directly by 2-byte transpose DMAs (one per D-chunk)
            for dc in range(8):
                nc.sync.dma_start(out=xT[:, dc, :],
                                  in_=xkv_d[:, dc * 128:(dc + 1) * 128],
                                  transpose=True)
            nc.sync.dma_start(out=msk_sb, in_=msk_d.ap())
            nc.sync.dma_start(out=wv_sb,
                              in_=wv_d.ap().rearrange("(c p) d -> p c d", p=128))
            nc.sync.dma_start(out=wk_sb,
                              in_=wk_d.ap().rearrange("(c p) d -> p c d", p=128))
            nc.sync.dma_start(out=wq_sb,
                              in_=wq_d.ap().rearrange("(c p) d -> p c d", p=128))
            nc.sync.dma_start(out=wo_sb,
                              in_=wo_d.ap().rearrange("(c p) d -> p c d", p=128))
            make_identity(nc, ident)
            nc.vector.memset(ones64, 1.0)

            with tc.tile_pool(name="pst", bufs=2, space="PSUM") as pst, \
                 tc.tile_pool(name="psp", bufs=2, space="PSUM") as psp:

                # ---- phase C: build gT (xT comes straight from transpose DMA)
                for gt_i in range(2):
                    for dc in range(8):
                        pt = pst.tile([128, 128], bf16, tag="tp")
                        nc.tensor.transpose(
                            pt, gnat[:, gt_i, dc * 128:(dc + 1) * 128], ident)
                        nc.scalar.copy(gT[:, dc, gt_i * 128:(gt_i + 1) * 128], pt)

                # ---- phase D: projections ----
                # kT [2-head chunk oc, kv tokens]
                for oc in range(8):
                    for ts_, te in ((0, 512), (512, 1024), (1024, 1280)):
                        pp = psp.tile([128, 512], f32, tag="pj")
                        for dc in range(8):
                            nc.tensor.matmul(
                                pp[:, :te - ts_],
                                wk_sb[:, dc, oc * 128:(oc + 1) * 128],
                                xT[:, dc, ts_:te],
                                start=(dc == 0), stop=(dc == 7))
                        nc.vector.tensor_copy(kT[:, oc, ts_:te], pp[:, :te - ts_])
                # qT (q tokens = xT kv-rows 128..1152), Wq pre-scaled by 1/8
                for oc in range(8):
                    for tch in range(2):
                        ts_ = 128 + tch * 512
                        pp = psp.tile([128, 512], f32, tag="pj")
                        for dc in range(8):
                            nc.tensor.matmul(
                                pp,
                                wq_sb[:, dc, oc * 128:(oc + 1) * 128],
                                xT[:, dc, ts_:ts_ + 512],
                                start=(dc == 0), stop=(dc == 7))
                        nc.vector.tensor_copy(
                            qT[:, oc, tch * 512:(tch + 1) * 512], pp)
                # side kT
                for oc in range(8):
                    pp = psp.tile([128, 512], f32, tag="pj")
                    for dc in range(8):
                        nc.tensor.matmul(
                            pp[:, :G],
                            wk_sb[:, dc, oc * 128:(oc + 1) * 128],
                            gT[:, dc, :],
                            start=(dc == 0), stop=(dc == 7))
                    nc.vector.tensor_copy(skT[:, oc, :], pp[:, :G])
                # v natural (augmented with ones column per head)
                for t in range(10):
                    for j in range(2):
                        pp = psp.tile([128, 512], f32, tag="pj")
                        for dc in range(8):
                            nc.tensor.matmul(
                                pp,
                                xT[:, dc, t * 128:(t + 1) * 128],
                                wv_sb[:, dc, 512 * j:512 * (j + 1)],
                                start=(dc == 0), stop=(dc == 7))
                        nc.scalar.copy(
                            vaug[:, t, 8 * j:8 * (j + 1), 0:DH],
                            pp.rearrange("p (h d) -> p h d", h=8))
                    nc.vector.memset(vaug[:, t, :, DH:DH + 1], 1.0)
                # side v (augmented)
                for gt_i in range(2):
                    for j in range(2):
                        pp = psp.tile([128, 512], f32, tag="pj")
                        for dc in range(8):
                            nc.tensor.matmul(
                                pp,
                                gT[:, dc, gt_i * 128:(gt_i + 1) * 128],
                                wv_sb[:, dc, 512 * j:512 * (j + 1)],
                                start=(dc == 0), stop=(dc == 7))
                        nc.scalar.copy(
                            svaug[:, gt_i, 8 * j:8 * (j + 1), 0:DH],
                            pp.rearrange("p (h d) -> p h d", h=8))
                    nc.vector.memset(svaug[:, gt_i, :, DH:DH + 1], 1.0)

            # ---- phase E: attention ----
            yTf = per.tile([128, 8, TOK], bf16, tag="wq_yT")
            stage_o = per.tile([64, 8, TOK], bf16, tag="xT_st")
            with tc.tile_pool(name="plg", bufs=3, space="PSUM") as plg, \
                 tc.tile_pool(name="psg2", bufs=2, space="PSUM") as psg2, \
                 tc.tile_pool(name="pyt", bufs=2, space="PSUM") as pyt, \
                 tc.tile_pool(name="pbc", bufs=1, space="PSUM") as pbc:
                for h in range(H):
                    oc, r0 = h // 2, 64 * (h % 2)
                    for nh in range(2):          # half = 4 blocks = 512 q
                        q4 = qT[r0:r0 + 64, oc, nh * 512:(nh + 1) * 512]
                        # side QK batched over the 4 blocks (N=512)
                        sgs = [psg2.tile([128, 512], f32, tag="sg",
                                         name=f"sg{h}_{nh}_{g}")
                               for g in range(2)]
                        us = att.tile([128, 2, 512], bf16, tag="us")
                        for g in range(2):
                            nc.tensor.matmul(
                                sgs[g],
                                skT[r0:r0 + 64, oc, g * 128:(g + 1) * 128],
                                q4, start=True, stop=True)
                            nc.scalar.activation(us[:, g, :], sgs[g], Exp)
                        # local QK + exp + mask per block
                        uls = []
                        for i in range(4):
                            n = nh * 4 + i
                            qs = qT[r0:r0 + 64, oc, n * 128:(n + 1) * 128]
                            lg = plg.tile([128, 3, 128], f32, tag="lg",
                                          name=f"lg{h}_{n}")
                            for c in range(3):
                                nc.tensor.matmul(
                                    lg[:, c, :],
                                    kT[r0:r0 + 64, oc,
                                       (n + c) * 128:(n + c + 1) * 128],
                                    qs, start=True, stop=True)
                            ul = att.tile([128, 3, 128], bf16, tag="ul", bufs=6,
                                          name=f"ul{h}_{n}")
                            nc.scalar.activation(ul, lg, Exp)
                            nc.vector.tensor_mul(ul, ul, msk_sb[:, n, :, :])
                            uls.append(ul)
                        # PV for 4 blocks into one [65, 512] psum
                        yt = pyt.tile([65, 512], f32, tag="yt")
                        for i in range(4):
                            n = nh * 4 + i
                            for c in range(3):
                                # start=True clears the whole PSUM tile, so
                                # only the very first matmul of the group may
                                # set it; per-element has_written handles the
                                # first write of each column slice.
                                nc.tensor.matmul(
                                    yt[:, i * 128:(i + 1) * 128],
                                    vaug[:, n + c, h, :], uls[i][:, c, :],
                                    start=(i == 0 and c == 0), stop=False)
                        for g in range(2):
                            nc.tensor.matmul(
                                yt, svaug[:, g, h, :], us[:, g, :],
                                start=False, stop=(g == 1))
                        # normalize: bcast (denom+1), lane-parallel recip, mul
                        rc = att.tile([128, 512], bf16, tag="rc", bufs=3)
                        nc.vector.tensor_scalar_add(rc[64:65, :], yt[64:65, :],
                                                    1.0)
                        bc = pbc.tile([64, 512], f32, tag="bc")
                        nc.tensor.matmul(bc, ones64[64:65, :], rc[64:65, :],
                                         start=True, stop=True)
                        rcb = att.tile([64, 512], bf16, tag="rcb", bufs=3)
                        with nc.allow_low_precision(reason="bf16 softmax recip"):
                            nc.vector.reciprocal(rcb, bc)
                        dst = (yTf[0:64, oc, nh * 512:(nh + 1) * 512]
                               if h % 2 == 0 else
                               stage_o[0:64, oc, nh * 512:(nh + 1) * 512])
                        nc.vector.tensor_mul(dst, yt[0:64, :], rcb)
                # shift odd heads up to partitions 64..127 (one big DMA)
                nc.sync.dma_start(out=yTf[64:128, :, :], in_=stage_o[0:64, :, :])

            # ---- phase F: output projection ----
            with tc.tile_pool(name="pso", bufs=2, space="PSUM") as pso:
                for tt in range(8):
                    ot = osb.tile([128, D], f32, tag="ot")
                    for j in range(2):
                        pp = pso.tile([128, 512], f32, tag="po")
                        for oc in range(8):
                            nc.tensor.matmul(
                                pp,
                                yTf[:, oc, tt * 128:(tt + 1) * 128],
                                wo_sb[:, oc, 512 * j:512 * (j + 1)],
                                start=(oc == 0), stop=(oc == 7))
                        nc.vector.tensor_copy(ot[:, 512 * j:512 * (j + 1)], pp)
                    nc.sync.dma_start(out=out_d[tt * 128:(tt + 1) * 128, :],
                                      in_=ot)

    nc.compile()
    return nc


def _host_inputs(x, Wq, Wk, Wv, Wo):
    """Build the 8 per-core input maps (all numpy, bf16 where device expects)."""
    xbf = x.astype(BF16)
    wq = (Wq.reshape(D, D).astype(np.float32) / np.sqrt(DH)).astype(BF16)
    wk = Wk.reshape(D, D).astype(BF16)
    wv = Wv.reshape(D, D).astype(BF16)
    wo = Wo.reshape(D, D).astype(BF16)

    # per-example side aggregates (sum of x over 16-token groups), f32 sum
    g_all = x.reshape(B, G, TPB, D).sum(2).astype(BF16)

    in_maps = []
    for c in range(N_CORES):
        b, s = c // 4, c % 4
        S0 = s * TOK
        blk0 = S0 // BL
        xkv = np.zeros((KV, D), BF16)
        a0 = S0 - BL
        lo, hi = max(a0, 0), min(a0 + KV, L)
        xkv[lo - a0:hi - a0] = xbf[b, lo:hi]
        # maskT[k, n, c, q]: local-window validity, transposed
        k_ = np.arange(BL)[:, None, None, None]
        n_ = np.arange(NB)[None, :, None, None]
        c_ = np.arange(3)[None, None, :, None]
        q_ = np.arange(BL)[None, None, None, :]
        rel = (c_ * BL + k_) - BL - q_
        kpos = (blk0 + n_ - 1) * BL + c_ * BL + k_
        valid = (np.abs(rel) <= BL - 1) & (kpos >= 0) & (kpos < L)
        in_maps.append({
            "xkv": xkv,
            "g": g_all[b],
            "wq": wq, "wk": wk, "wv": wv, "wo": wo,
            "maskT": valid.astype(BF16),
        })
    return in_maps


_RUNNER = None


def _make_runner(nc):
    """Build the PJRT executable once; returns fn(in_maps) -> per-core outs.

    Mirrors concourse.bass2jax.run_bass_via_pjrt, but caches the jitted
    shard_map callable so repeat kernel() calls skip retrace/recompile.
    """
    import jax
    import numpy as _np
    from jax.sharding import Mesh, PartitionSpec
    from jax.experimental.shard_map import shard_map
    import concourse.mybir as mybir
    from concourse import bass2jax

    bass2jax.install_neuronx_cc_hook()
    partition_name = (nc.partition_id_tensor.name
                      if nc.partition_id_tensor else None)
    in_names, out_names, out_avals = [], [], []
    for alloc in nc.m.functions[0].allocations:
        if not isinstance(alloc, mybir.MemoryLocationSet):
            continue
        name = alloc.memorylocations[0].name
        if alloc.kind == "ExternalInput":
            if name != partition_name:
                in_names.append(name)
        elif alloc.kind == "ExternalOutput":
            out_avals.append(jax.core.ShapedArray(
                tuple(alloc.tensor_shape), mybir.dt.np(alloc.dtype)))
            out_names.append(name)
    n_params = len(in_names)
    all_names = in_names + out_names
    if partition_name is not None:
        all_names.append(partition_name)
    donate = tuple(range(n_params, n_params + len(out_names)))

    def _body(*args):
        operands = list(args)
        if partition_name is not None:
            operands.append(bass2jax.partition_id_tensor())
        return tuple(bass2jax._bass_exec_p.bind(
            *operands, out_avals=tuple(out_avals), in_names=tuple(all_names),
            out_names=tuple(out_names), lowering_input_output_aliases=(),
            sim_require_finite=True, sim_require_nnan=True, nc=nc))

    devices = jax.devices()[:N_CORES]
    mesh = Mesh(_np.asarray(devices), ("core",))
    specs = (PartitionSpec("core"),) * (n_params + len(out_names))
    sharded = jax.jit(
        shard_map(_body, mesh=mesh, in_specs=specs,
                  out_specs=(PartitionSpec("core"),) * len(out_names),
                  check_rep=False),
        donate_argnums=donate, keep_unused=True)

    def run(in_maps):
        concat_in = [
            _np.concatenate([_np.asarray(in_maps[c][k]) for c in range(N_CORES)],
                            axis=0)
            for k in in_names
        ]
        concat_zeros = [_np.zeros((N_CORES * a.shape[0], *a.shape[1:]), a.dtype)
                        for a in out_avals]
        outs = sharded(*concat_in, *concat_zeros)
        return [
            {k: _np.asarray(outs[i]).reshape(N_CORES, *out_avals[i].shape)[c]
             for i, k in enumerate(out_names)}
            for c in range(N_CORES)
        ]

    return run


def kernel(x, Wq, Wk, Wv, Wo):
    global _PROG, _RUNNER
    if _RUNNER is None:
        _PROG = _build_program()
        _RUNNER = _make_runner(_PROG)
    in_maps = _host_inputs(np.asarray(x, np.float32), np.asarray(Wq, np.float32),
                           np.asarray(Wk, np.float32), np.asarray(Wv, np.float32),
                           np.asarray(Wo, np.float32))
    results = _RUNNER(in_maps)
    out = np.empty((B, L, D), np.float32)
    for c in range(N_CORES):
        b, s = c // 4, c % 4
        out[b, s * TOK:(s + 1) * TOK] = results[c]["out"]
    return out


# revision 23
# speedup vs baseline: 14575.6255x; 1.0268x over previous
"""ETC transient-global self-attention on 8 TRN2 NeuronCores.

Sharding: sequence-parallel. Core c handles example b = c//4, token rows
[1024*(c%4), 1024*(c%4+1)). Each core computes its q/k/v projections (k/v
with a 1-block halo), the per-example global (side) aggregates from the
full example, local+global attention, and the output projection for its
token rows. No cross-core communication; the host stacks the 8 row-slices.

Shapes (hardcoded from the problem spec):
  x  [2, 4096, 1024], Wq/Wk/Wv [1024, 16, 64], Wo [16, 64, 1024]
  block_len 128, 32 blocks, TOKENS_PER_BLOCK 16 -> G = 256 side tokens.

On-device layout notes:
  - everything runs in bf16 on the PE (f32 accumulate in PSUM).
  - attention logits are computed TRANSPOSED ([keys, q]) so that exp() is
    orientation-agnostic and PV contracts keys on the partition dim with no
    probs transpose. Softmax denominators come from a ones-column appended
    to v (PSUM row 64); the reference's extra-logit softmax needs no
    max-subtraction since logits are O(1) and masked entries multiply to 0.
"""

import numpy as np
import ml_dtypes

B, L, D, H, DH = 2, 4096, 1024, 16, 64
BL = 128                 # block length
NBLK = 32                # total blocks
G = 256                  # side (global) tokens
TPB = 16                 # tokens per side block
N_CORES = 8
NB = 8                   # blocks per core
TOK = NB * BL            # 1024 q tokens per core
KV = TOK + 2 * BL        # 1280 kv tokens (1-block halo each side)
BF16 = ml_dtypes.bfloat16

_PROG = None


def _build_program():
    import concourse.bass as bass
    import concourse.mybir as mybir
    import concourse.tile as tile
    from concourse import bacc
    from concourse.masks import make_identity

    dt = mybir.dt
    f32, bf16 = dt.float32, dt.bfloat16

    nc = bacc.Bacc("TRN2", target_bir_lowering=False, debug=False,
                   num_devices=N_CORES)

    xkv_d = nc.declare_dram_parameter("xkv", [KV, D], bf16, isOutput=False)
    g_d = nc.declare_dram_parameter("g", [G, D], bf16, isOutput=False)
    wq_d = nc.declare_dram_parameter("wq", [D, D], bf16, isOutput=False)
    wk_d = nc.declare_dram_parameter("wk", [D, D], bf16, isOutput=False)
    wv_d = nc.declare_dram_parameter("wv", [D, D], bf16, isOutput=False)
    wo_d = nc.declare_dram_parameter("wo", [D, D], bf16, isOutput=False)
    msk_d = nc.declare_dram_parameter("maskT", [BL, NB, 3, BL], bf16,
                                      isOutput=False)
    out_d = nc.declare_dram_parameter("out", [TOK, D], f32, isOutput=True)

    Exp = mybir.ActivationFunctionType.Exp
    Cpy = mybir.ActivationFunctionType.Copy

    with tile.TileContext(nc) as tc:
        with (
            tc.tile_pool(name="per", bufs=1) as per,
            tc.tile_pool(name="strm", bufs=3) as strm,
            tc.tile_pool(name="att", bufs=2) as att,
            tc.tile_pool(name="osb", bufs=2) as osb,
        ):
            # ---- persistent SBUF tiles ----
            wk_sb = per.tile([128, 8, D], bf16, tag="wk")
            wv_sb = per.tile([128, 8, D], bf16, tag="wv")
            wo_sb = per.tile([128, 8, D], bf16, tag="wo")
            wq_sb = per.tile([128, 8, D], bf16, tag="wq_yT")   # dies -> yTf
            msk_sb = per.tile([128, NB, 3, BL], bf16, tag="msk")
            ident = per.tile([128, 128], bf16, tag="ident")
            ones64 = per.tile([128, 64], bf16, tag="ones64")
            gnat = per.tile([128, 2, D], bf16, tag="gnat")
            gT = per.tile([128, 8, G], bf16, tag="gT")
            skT = per.tile([128, 8, G], bf16, tag="skT")
            svaug = per.tile([128, 2, H, DH + 1], bf16, tag="svaug")
            qT = per.tile([128, 8, TOK], bf16, tag="qT")
            kT = per.tile([128, 8, KV], bf16, tag="kT")
            vaug = per.tile([128, 10, H, DH + 1], bf16, tag="vaug")
            xT = per.tile([128, 8, KV], bf16, tag="xT_st")     # dies -> stage_o

            # DMA issue order matters for ramp-up: inputs feeding the first
            # compute (g + xT transpose loads) go first; weights arrive
            # later, in first-use order.
            nc.sync.dma_start(out=gnat,
                              in_=g_d.ap().rearrange("(t p) d -> p t d", p=128))
            # xT built directly by 2-byte transpose DMAs (one per D-chunk)
            for dc in range(8):
                nc.sync.dma_start(out=xT[:, dc, :],
                                  in_=xkv_d[:, dc * 128:(dc + 1) * 128],
                                  transpose=True)
            nc.sync.dma_start(out=wk_sb,
                              in_=wk_d.ap().rearrange("(c p) d -> p c d", p=128))
            nc.sync.dma_start(out=wq_sb,
                              in_=wq_d.ap().rearrange("(c p) d -> p c d", p=128))
            nc.sync.dma_start(out=wv_sb,
                              in_=wv_d.ap().rearrange("(c p) d -> p c d", p=128))
            nc.sync.dma_start(out=msk_sb, in_=msk_d.ap())
            nc.sync.dma_start(out=wo_sb,
                              in_=wo_d.ap().rearrange("(c p) d -> p c d", p=128))
            make_identity(nc, ident)
            nc.vector.memset(ones64, 1.0)

            with tc.tile_pool(name="pst", bufs=2, space="PSUM") as pst, \
                 tc.tile_pool(name="psp", bufs=2, space="PSUM") as psp:

                # ---- phase C: build gT (xT comes straight from transpose DMA)
                for gt_i in range(2):
                    for dc in range(8):
                        pt = pst.tile([128, 128], bf16, tag="tp")
                        nc.tensor.transpose(
                            pt, gnat[:, gt_i, dc * 128:(dc + 1) * 128], ident)
                        nc.scalar.copy(gT[:, dc, gt_i * 128:(gt_i + 1) * 128], pt)

                # ---- phase D: projections ----
                # kT [2-head chunk oc, kv tokens]
                for oc in range(8):
                    for ts_, te in ((0, 512), (512, 1024), (1024, 1280)):
                        pp = psp.tile([128, 512], f32, tag="pj")
                        for dc in range(8):
                            nc.tensor.matmul(
                                pp[:, :te - ts_],
                                wk_sb[:, dc, oc * 128:(oc + 1) * 128],
                                xT[:, dc, ts_:te],
                                start=(dc == 0), stop=(dc == 7))
                        nc.vector.tensor_copy(kT[:, oc, ts_:te], pp[:, :te - ts_])
                # qT (q tokens = xT kv-rows 128..1152), Wq pre-scaled by 1/8
                for oc in range(8):
                    for tch in range(2):
                        ts_ = 128 + tch * 512
                        pp = psp.tile([128, 512], f32, tag="pj")
                        for dc in range(8):
                            nc.tensor.matmul(
                                pp,
                                wq_sb[:, dc, oc * 128:(oc + 1) * 128],
                                xT[:, dc, ts_:ts_ + 512],
                                start=(dc == 0), stop=(dc == 7))
                        nc.vector.tensor_copy(
                            qT[:, oc, tch * 512:(tch + 1) * 512], pp)
                # side kT
                for oc in range(8):
                    pp = psp.tile([128, 512], f32, tag="pj")
                    for dc in range(8):
                        nc.tensor.matmul(
                            pp[:, :G],
                            wk_sb[:, dc, oc * 128:(oc + 1) * 128],
                            gT[:, dc, :],
                            start=(dc == 0), stop=(dc == 7))
                    nc.vector.tensor_copy(skT[:, oc, :], pp[:, :G])
                # v natural (augmented with ones column per head)
                for t in range(10):
                    for j in range(2):
                        pp = psp.tile([128, 512], f32, tag="pj")
                        for dc in range(8):
                            nc.tensor.matmul(
                                pp,
                                xT[:, dc, t * 128:(t + 1) * 128],
                                wv_sb[:, dc, 512 * j:512 * (j + 1)],
                                start=(dc == 0), stop=(dc == 7))
                        nc.scalar.copy(
                            vaug[:, t, 8 * j:8 * (j + 1), 0:DH],
                            pp.rearrange("p (h d) -> p h d", h=8))
                    nc.vector.memset(vaug[:, t, :, DH:DH + 1], 1.0)
                # side v (augmented)
                for gt_i in range(2):
                    for j in range(2):
                        pp = psp.tile([128, 512], f32, tag="pj")
                        for dc in range(8):
                            nc.tensor.matmul(
                                pp,
                                gT[:, dc, gt_i * 128:(gt_i + 1) * 128],
                                wv_sb[:, dc, 512 * j:512 * (j + 1)],
                                start=(dc == 0), stop=(dc == 7))
                        nc.scalar.copy(
                            svaug[:, gt_i, 8 * j:8 * (j + 1), 0:DH],
                            pp.rearrange("p (h d) -> p h d", h=8))
                    nc.vector.memset(svaug[:, gt_i, :, DH:DH + 1], 1.0)

            # ---- phase E: attention ----
            yTf = per.tile([128, 8, TOK], bf16, tag="wq_yT")
            stage_o = per.tile([64, 8, TOK], bf16, tag="xT_st")
            with tc.tile_pool(name="plg", bufs=3, space="PSUM") as plg, \
                 tc.tile_pool(name="psg2", bufs=2, space="PSUM") as psg2, \
                 tc.tile_pool(name="pyt", bufs=2, space="PSUM") as pyt, \
                 tc.tile_pool(name="pbc", bufs=1, space="PSUM") as pbc:
                for h in range(H):
                    oc, r0 = h // 2, 64 * (h % 2)
                    for nh in range(2):          # half = 4 blocks = 512 q
                        q4 = qT[r0:r0 + 64, oc, nh * 512:(nh + 1) * 512]
                        # side QK batched over the 4 blocks (N=512)
                        sgs = [psg2.tile([128, 512], f32, tag="sg",
                                         name=f"sg{h}_{nh}_{g}")
                               for g in range(2)]
                        us = att.tile([128, 2, 512], bf16, tag="us")
                        for g in range(2):
                            nc.tensor.matmul(
                                sgs[g],
                                skT[r0:r0 + 64, oc, g * 128:(g + 1) * 128],
                                q4, start=True, stop=True)
                            nc.scalar.activation(us[:, g, :], sgs[g], Exp)
                        # local QK + exp + mask per block
                        uls = []
                        for i in range(4):
                            n = nh * 4 + i
                            qs = qT[r0:r0 + 64, oc, n * 128:(n + 1) * 128]
                            lg = plg.tile([128, 3, 128], f32, tag="lg",
                                          name=f"lg{h}_{n}")
                            for c in range(3):
                                nc.tensor.matmul(
                                    lg[:, c, :],
                                    kT[r0:r0 + 64, oc,
                                       (n + c) * 128:(n + c + 1) * 128],
                                    qs, start=True, stop=True)
                            ul = att.tile([128, 3, 128], bf16, tag="ul", bufs=6,
                                          name=f"ul{h}_{n}")
                            nc.scalar.activation(ul, lg, Exp)
                            nc.vector.tensor_mul(ul, ul, msk_sb[:, n, :, :])
                            uls.append(ul)
                        # PV for 4 blocks into one [65, 512] psum
                        yt = pyt.tile([65, 512], f32, tag="yt")
                        for i in range(4):
                            n = nh * 4 + i
                            for c in range(3):
                                # start=True clears the whole PSUM tile, so
                                # only the very first matmul of the group may
                                # set it; per-element has_written handles the
                                # first write of each column slice.
                                nc.tensor.matmul(
                                    yt[:, i * 128:(i + 1) * 128],
                                    vaug[:, n + c, h, :], uls[i][:, c, :],
                                    start=(i == 0 and c == 0), stop=False)
                        for g in range(2):
                            nc.tensor.matmul(
                                yt, svaug[:, g, h, :], us[:, g, :],
                                start=False, stop=(g == 1))
                        # normalize: bcast (denom+1), lane-parallel recip, mul
                        rc = att.tile([128, 512], bf16, tag="rc", bufs=3)
                        nc.vector.tensor_scalar_add(rc[64:65, :], yt[64:65, :],
                                                    1.0)
                        bc = pbc.tile([64, 512], f32, tag="bc")
                        nc.tensor.matmul(bc, ones64[64:65, :], rc[64:65, :],
                                         start=True, stop=True)
                        rcb = att.tile([64, 512], bf16, tag="rcb", bufs=3)
                        with nc.allow_low_precision(reason="bf16 softmax recip"):
                            nc.vector.reciprocal(rcb, bc)
                        dst = (yTf[0:64, oc, nh * 512:(nh + 1) * 512]
                               if h % 2 == 0 else
                               stage_o[0:64, oc, nh * 512:(nh + 1) * 512])
                        nc.vector.tensor_mul(dst, yt[0:64, :], rcb)
                # shift odd heads up to partitions 64..127 (one big DMA)
                nc.sync.dma_start(out=yTf[64:128, :, :], in_=stage_o[0:64, :, :])

            # ---- phase F: output projection ----
            with tc.tile_pool(name="pso", bufs=2, space="PSUM") as pso:
                for tt in range(8):
                    ot = osb.tile([128, D], f32, tag="ot")
                    for j in range(2):
                        pp = pso.tile([128, 512], f32, tag="po")
                        for oc in range(8):
                            nc.tensor.matmul(
                                pp,
                                yTf[:, oc, tt * 128:(tt + 1) * 128],
                                wo_sb[:, oc, 512 * j:512 * (j + 1)],
                                start=(oc == 0), stop=(oc == 7))
                        nc.vector.tensor_copy(ot[:, 512 * j:512 * (j + 1)], pp)
                    nc.sync.dma_start(out=out_d[tt * 128:(tt + 1) * 128, :],
                                      in_=ot)

    nc.compile()
    return nc


def _host_inputs(x, Wq, Wk, Wv, Wo):
    """Build the 8 per-core input maps (all numpy, bf16 where device expects)."""
    xbf = x.astype(BF16)
    wq = (Wq.reshape(D, D).astype(np.float32) / np.sqrt(DH)).astype(BF16)
    wk = Wk.reshape(D, D).astype(BF16)
    wv = Wv.reshape(D, D).astype(BF16)
    wo = Wo.reshape(D, D).astype(BF16)

    # per-example side aggregates (sum of x over 16-token groups), f32 sum
    g_all = x.reshape(B, G, TPB, D).sum(2).astype(BF16)

    in_maps = []
    for c in range(N_CORES):
        b, s = c // 4, c % 4
        S0 = s * TOK
        blk0 = S0 // BL
        xkv = np.zeros((KV, D), BF16)
        a0 = S0 - BL
        lo, hi = max(a0, 0), min(a0 + KV, L)
        xkv[lo - a0:hi - a0] = xbf[b, lo:hi]
        # maskT[k, n, c, q]: local-window validity, transposed
        k_ = np.arange(BL)[:, None, None, None]
        n_ = np.arange(NB)[None, :, None, None]
        c_ = np.arange(3)[None, None, :, None]
        q_ = np.arange(BL)[None, None, None, :]
        rel = (c_ * BL + k_) - BL - q_
        kpos = (blk0 + n_ - 1) * BL + c_ * BL + k_
        valid = (np.abs(rel) <= BL - 1) & (kpos >= 0) & (kpos < L)
        in_maps.append({
            "xkv": xkv,
            "g": g_all[b],
            "wq": wq, "wk": wk, "wv": wv, "wo": wo,
            "maskT": valid.astype(BF16),
        })
    return in_maps


_RUNNER = None


def _make_runner(nc):
    """Build the PJRT executable once; returns fn(in_maps) -> per-core outs.

    Mirrors concourse.bass2jax.run_bass_via_pjrt, but caches the jitted
    shard_map callable so repeat kernel() calls skip retrace/recompile.
    """
    import jax
    import numpy as _np
    from jax.sharding import Mesh, PartitionSpec
    from jax.experimental.shard_map import shard_map
    import concourse.mybir as mybir
    from concourse import bass2jax

    bass2jax.install_neuronx_cc_hook()
    partition_name = (nc.partition_id_tensor.name
                      if nc.partition_id_tensor else None)
    in_names, out_names, out_avals = [], [], []
    for alloc in nc.m.functions[0].allocations:
        if not isinstance(alloc, mybir.MemoryLocationSet):
            continue
        name = alloc.memorylocations[0].name
        if alloc.kind == "ExternalInput":
            if name != partition_name:
                in_names.append(name)
        elif alloc.kind == "ExternalOutput":
            out_avals.append(jax.core.ShapedArray(
                tuple(alloc.tensor_shape), mybir.dt.np(alloc.dtype)))
            out_names.append(name)
    n_params = len(in_names)
    all_names = in_names + out_names
    if partition_name is not None:
        all_names.append(partition_name)
    donate = tuple(range(n_params, n_params + len(out_names)))

    def _body(*args):
        operands = list(args)
        if partition_name is not None:
            operands.append(bass2jax.partition_id_tensor())
        return tuple(bass2jax._bass_exec_p.bind(
            *operands, out_avals=tuple(out_avals), in_names=tuple(all_names),
            out_names=tuple(out_names), lowering_input_output_aliases=(),
            sim_require_finite=True, sim_require_nnan=True, nc=nc))

    devices = jax.devices()[:N_CORES]
    mesh = Mesh(_np.asarray(devices), ("core",))
    specs = (PartitionSpec("core"),) * (n_params + len(out_names))
    sharded = jax.jit(
        shard_map(_body, mesh=mesh, in_specs=specs,
                  out_specs=(PartitionSpec("core"),) * len(out_names),
                  check_rep=False),
        donate_argnums=donate, keep_unused=True)

    def run(in_maps):
        concat_in = [
            _np.concatenate([_np.asarray(in_maps[c][k]) for c in range(N_CORES)],
                            axis=0)
            for k in in_names
        ]
        concat_zeros = [_np.zeros((N_CORES * a.shape[0], *a.shape[1:]), a.dtype)
                        for a in out_avals]
        outs = sharded(*concat_in, *concat_zeros)
        return [
            {k: _np.asarray(outs[i]).reshape(N_CORES, *out_avals[i].shape)[c]
             for i, k in enumerate(out_names)}
            for c in range(N_CORES)
        ]

    return run


def kernel(x, Wq, Wk, Wv, Wo):
    global _PROG, _RUNNER
    if _RUNNER is None:
        _PROG = _build_program()
        _RUNNER = _make_runner(_PROG)
    in_maps = _host_inputs(np.asarray(x, np.float32), np.asarray(Wq, np.float32),
                           np.asarray(Wk, np.float32), np.asarray(Wv, np.float32),
                           np.asarray(Wo, np.float32))
    results = _RUNNER(in_maps)
    out = np.empty((B, L, D), np.float32)
    for c in range(N_CORES):
        b, s = c // 4, c % 4
        out[b, s * TOK:(s + 1) * TOK] = results[c]["out"]
    return out


# revision 30
# speedup vs baseline: 15170.6174x; 1.0408x over previous
"""ETC transient-global self-attention on 8 TRN2 NeuronCores.

Sharding: sequence-parallel. Core c handles example b = c//4, token rows
[1024*(c%4), 1024*(c%4+1)). Each core computes its q/k/v projections (k/v
with a 1-block halo), the per-example global (side) aggregates from the
full example, local+global attention, and the output projection for its
token rows. No cross-core communication; the host stacks the 8 row-slices.

Shapes (hardcoded from the problem spec):
  x  [2, 4096, 1024], Wq/Wk/Wv [1024, 16, 64], Wo [16, 64, 1024]
  block_len 128, 32 blocks, TOKENS_PER_BLOCK 16 -> G = 256 side tokens.

On-device layout notes:
  - everything runs in bf16 on the PE (f32 accumulate in PSUM).
  - attention logits are computed TRANSPOSED ([keys, q]) so that exp() is
    orientation-agnostic and PV contracts keys on the partition dim with no
    probs transpose. Softmax denominators come from a ones-column appended
    to v (PSUM row 64); the reference's extra-logit softmax needs no
    max-subtraction since logits are O(1) and masked entries multiply to 0.
"""

import numpy as np
import ml_dtypes

B, L, D, H, DH = 2, 4096, 1024, 16, 64
BL = 128                 # block length
NBLK = 32                # total blocks
G = 256                  # side (global) tokens
TPB = 16                 # tokens per side block
N_CORES = 8
NB = 8                   # blocks per core
TOK = NB * BL            # 1024 q tokens per core
KV = TOK + 2 * BL        # 1280 kv tokens (1-block halo each side)
BF16 = ml_dtypes.bfloat16

_PROG = None


def _build_program():
    import concourse.bass as bass
    import concourse.mybir as mybir
    import concourse.tile as tile
    from concourse import bacc
    from concourse.masks import make_identity

    dt = mybir.dt
    f32, bf16 = dt.float32, dt.bfloat16

    nc = bacc.Bacc("TRN2", target_bir_lowering=False, debug=False,
                   num_devices=N_CORES)

    xkv_d = nc.declare_dram_parameter("xkv", [KV, D], bf16, isOutput=False)
    g_d = nc.declare_dram_parameter("g", [G, D], bf16, isOutput=False)
    wq_d = nc.declare_dram_parameter("wq", [D, D], bf16, isOutput=False)
    wk_d = nc.declare_dram_parameter("wk", [D, D], bf16, isOutput=False)
    wv_d = nc.declare_dram_parameter("wv", [D, D], bf16, isOutput=False)
    wo_d = nc.declare_dram_parameter("wo", [D, D], bf16, isOutput=False)
    msk_d = nc.declare_dram_parameter("maskT", [BL, NB, 3, BL], bf16,
                                      isOutput=False)
    out_d = nc.declare_dram_parameter("out", [TOK, D], f32, isOutput=True)

    Exp = mybir.ActivationFunctionType.Exp
    Cpy = mybir.ActivationFunctionType.Copy

    with tile.TileContext(nc) as tc:
        with (
            tc.tile_pool(name="per", bufs=1) as per,
            tc.tile_pool(name="strm", bufs=3) as strm,
            tc.tile_pool(name="att", bufs=2) as att,
            tc.tile_pool(name="osb", bufs=2) as osb,
        ):
            # ---- persistent SBUF tiles ----
            wk_sb = per.tile([128, 8, D], bf16, tag="wk")
            wv_sb = per.tile([128, 8, D], bf16, tag="wv")
            wo_sb = per.tile([128, 8, D], bf16, tag="wo")
            wq_sb = per.tile([128, 8, D], bf16, tag="wq_yT")   # dies -> yTf
            msk_sb = per.tile([128, NB, 3, BL], bf16, tag="msk")
            ident = per.tile([128, 128], bf16, tag="ident")
            ones64 = per.tile([128, 64], bf16, tag="ones64")
            gnat = per.tile([128, 2, D], bf16, tag="gnat")
            gT = per.tile([128, 8, G], bf16, tag="gT")
            skT = per.tile([128, 8, G], bf16, tag="skT")
            svaug = per.tile([128, 2, H, DH + 1], bf16, tag="svaug")
            qT = per.tile([128, 8, TOK], bf16, tag="qT")
            kT = per.tile([128, 8, KV], bf16, tag="kT")
            vaug = per.tile([128, 10, H, DH + 1], bf16, tag="vaug")
            xT = per.tile([128, 8, KV], bf16, tag="xT_st")     # dies -> stage_o

            # DMA issue order matters for ramp-up: wk/wq land first so the
            # first kT accumulation group can chase the xT transpose chunks
            # as they arrive instead of waiting for the whole queue.
            nc.sync.dma_start(out=wk_sb,
                              in_=wk_d.ap().rearrange("(c p) d -> p c d", p=128))
            nc.sync.dma_start(out=wq_sb,
                              in_=wq_d.ap().rearrange("(c p) d -> p c d", p=128))
            # xT built directly by 2-byte transpose DMAs (one per D-chunk)
            for dc in range(8):
                nc.sync.dma_start(out=xT[:, dc, :],
                                  in_=xkv_d[:, dc * 128:(dc + 1) * 128],
                                  transpose=True)
            nc.sync.dma_start(out=gnat,
                              in_=g_d.ap().rearrange("(t p) d -> p t d", p=128))
            nc.sync.dma_start(out=wv_sb,
                              in_=wv_d.ap().rearrange("(c p) d -> p c d", p=128))
            nc.sync.dma_start(out=msk_sb, in_=msk_d.ap())
            nc.sync.dma_start(out=wo_sb,
                              in_=wo_d.ap().rearrange("(c p) d -> p c d", p=128))
            make_identity(nc, ident)
            nc.vector.memset(ones64, 1.0)

            with tc.tile_pool(name="pst", bufs=2, space="PSUM") as pst, \
                 tc.tile_pool(name="psp", bufs=2, space="PSUM") as psp:

                # ---- phase C: build gT (xT comes straight from transpose DMA)
                for gt_i in range(2):
                    for dc in range(8):
                        pt = pst.tile([128, 128], bf16, tag="tp")
                        nc.tensor.transpose(
                            pt, gnat[:, gt_i, dc * 128:(dc + 1) * 128], ident)
                        nc.scalar.copy(gT[:, dc, gt_i * 128:(gt_i + 1) * 128], pt)

                # ---- phase D: projections ----
                # kT [2-head chunk oc, kv tokens]
                for oc in range(8):
                    for ts_, te in ((0, 512), (512, 1024), (1024, 1280)):
                        pp = psp.tile([128, 512], f32, tag="pj")
                        for dc in range(8):
                            nc.tensor.matmul(
                                pp[:, :te - ts_],
                                wk_sb[:, dc, oc * 128:(oc + 1) * 128],
                                xT[:, dc, ts_:te],
                                start=(dc == 0), stop=(dc == 7))
                        nc.vector.tensor_copy(kT[:, oc, ts_:te], pp[:, :te - ts_])
                # qT (q tokens = xT kv-rows 128..1152), Wq pre-scaled by 1/8
                for oc in range(8):
                    for tch in range(2):
                        ts_ = 128 + tch * 512
                        pp = psp.tile([128, 512], f32, tag="pj")
                        for dc in range(8):
                            nc.tensor.matmul(
                                pp,
                                wq_sb[:, dc, oc * 128:(oc + 1) * 128],
                                xT[:, dc, ts_:ts_ + 512],
                                start=(dc == 0), stop=(dc == 7))
                        nc.vector.tensor_copy(
                            qT[:, oc, tch * 512:(tch + 1) * 512], pp)
                # side kT
                for oc in range(8):
                    pp = psp.tile([128, 512], f32, tag="pj")
                    for dc in range(8):
                        nc.tensor.matmul(
                            pp[:, :G],
                            wk_sb[:, dc, oc * 128:(oc + 1) * 128],
                            gT[:, dc, :],
                            start=(dc == 0), stop=(dc == 7))
                    nc.vector.tensor_copy(skT[:, oc, :], pp[:, :G])
                # v natural (augmented with ones column per head)
                for t in range(10):
                    for j in range(2):
                        pp = psp.tile([128, 512], f32, tag="pj")
                        for dc in range(8):
                            nc.tensor.matmul(
                                pp,
                                xT[:, dc, t * 128:(t + 1) * 128],
                                wv_sb[:, dc, 512 * j:512 * (j + 1)],
                                start=(dc == 0), stop=(dc == 7))
                        nc.scalar.copy(
                            vaug[:, t, 8 * j:8 * (j + 1), 0:DH],
                            pp.rearrange("p (h d) -> p h d", h=8))
                    nc.vector.memset(vaug[:, t, :, DH:DH + 1], 1.0)
                # side v (augmented)
                for gt_i in range(2):
                    for j in range(2):
                        pp = psp.tile([128, 512], f32, tag="pj")
                        for dc in range(8):
                            nc.tensor.matmul(
                                pp,
                                gT[:, dc, gt_i * 128:(gt_i + 1) * 128],
                                wv_sb[:, dc, 512 * j:512 * (j + 1)],
                                start=(dc == 0), stop=(dc == 7))
                        nc.scalar.copy(
                            svaug[:, gt_i, 8 * j:8 * (j + 1), 0:DH],
                            pp.rearrange("p (h d) -> p h d", h=8))
                    nc.vector.memset(svaug[:, gt_i, :, DH:DH + 1], 1.0)

            # ---- phase E: attention ----
            yTf = per.tile([128, 8, TOK], bf16, tag="wq_yT")
            stage_o = per.tile([64, 8, TOK], bf16, tag="xT_st")
            with tc.tile_pool(name="plg", bufs=3, space="PSUM") as plg, \
                 tc.tile_pool(name="psg2", bufs=1, space="PSUM") as psg2, \
                 tc.tile_pool(name="pyt", bufs=2, space="PSUM") as pyt, \
                 tc.tile_pool(name="pbc", bufs=1, space="PSUM") as pbc:
                for h in range(H):
                    oc, r0 = h // 2, 64 * (h % 2)
                    for nh in range(2):          # half = 4 blocks = 512 q
                        q4 = qT[r0:r0 + 64, oc, nh * 512:(nh + 1) * 512]
                        # side QK batched over the 4 blocks (N=512), one exp
                        sg = psg2.tile([128, 2, 512], f32, tag="sg",
                                       name=f"sg{h}_{nh}")
                        us = att.tile([128, 2, 512], bf16, tag="us", bufs=3)
                        for g in range(2):
                            nc.tensor.matmul(
                                sg[:, g, :],
                                skT[r0:r0 + 64, oc, g * 128:(g + 1) * 128],
                                q4, start=True, stop=True)
                        nc.scalar.activation(us, sg, Exp)
                        # local QK + exp + mask per block
                        uls = []
                        for i in range(4):
                            n = nh * 4 + i
                            qs = qT[r0:r0 + 64, oc, n * 128:(n + 1) * 128]
                            lg = plg.tile([128, 3, 128], f32, tag="lg",
                                          name=f"lg{h}_{n}")
                            for c in range(3):
                                nc.tensor.matmul(
                                    lg[:, c, :],
                                    kT[r0:r0 + 64, oc,
                                       (n + c) * 128:(n + c + 1) * 128],
                                    qs, start=True, stop=True)
                            ul = att.tile([128, 3, 128], bf16, tag="ul", bufs=8,
                                          name=f"ul{h}_{n}")
                            nc.scalar.activation(ul, lg, Exp)
                            nc.vector.tensor_mul(ul, ul, msk_sb[:, n, :, :])
                            uls.append(ul)
                        # PV for 4 blocks into one [65, 512] psum
                        yt = pyt.tile([65, 512], f32, tag="yt")
                        for i in range(4):
                            n = nh * 4 + i
                            for c in range(3):
                                # start=True clears the whole PSUM tile, so
                                # only the very first matmul of the group may
                                # set it; per-element has_written handles the
                                # first write of each column slice.
                                nc.tensor.matmul(
                                    yt[:, i * 128:(i + 1) * 128],
                                    vaug[:, n + c, h, :], uls[i][:, c, :],
                                    start=(i == 0 and c == 0), stop=False)
                        for g in range(2):
                            nc.tensor.matmul(
                                yt, svaug[:, g, h, :], us[:, g, :],
                                start=False, stop=(g == 1))
                        # normalize: bcast (denom+1), lane-parallel recip, mul
                        rc = att.tile([128, 512], bf16, tag="rc", bufs=3)
                        nc.vector.tensor_scalar_add(rc[64:65, :], yt[64:65, :],
                                                    1.0)
                        bc = pbc.tile([64, 512], f32, tag="bc")
                        nc.tensor.matmul(bc, ones64[64:65, :], rc[64:65, :],
                                         start=True, stop=True)
                        rcb = att.tile([64, 512], bf16, tag="rcb", bufs=3)
                        with nc.allow_low_precision(reason="bf16 softmax recip"):
                            nc.vector.reciprocal(rcb, bc)
                        dst = (yTf[0:64, oc, nh * 512:(nh + 1) * 512]
                               if h % 2 == 0 else
                               stage_o[0:64, oc, nh * 512:(nh + 1) * 512])
                        nc.vector.tensor_mul(dst, yt[0:64, :], rcb)
                # shift odd heads up to partitions 64..127 (one big DMA)
                nc.sync.dma_start(out=yTf[64:128, :, :], in_=stage_o[0:64, :, :])

            # ---- phase F: output projection ----
            with tc.tile_pool(name="pso", bufs=2, space="PSUM") as pso:
                for tt in range(8):
                    ot = osb.tile([128, D], f32, tag="ot")
                    for j in range(2):
                        pp = pso.tile([128, 512], f32, tag="po")
                        for oc in range(8):
                            nc.tensor.matmul(
                                pp,
                                yTf[:, oc, tt * 128:(tt + 1) * 128],
                                wo_sb[:, oc, 512 * j:512 * (j + 1)],
                                start=(oc == 0), stop=(oc == 7))
                        nc.vector.tensor_copy(ot[:, 512 * j:512 * (j + 1)], pp)
                    nc.sync.dma_start(out=out_d[tt * 128:(tt + 1) * 128, :],
                                      in_=ot)

    nc.compile()
    return nc


def _host_inputs(x, Wq, Wk, Wv, Wo):
    """Build the 8 per-core input maps (all numpy, bf16 where device expects)."""
    xbf = x.astype(BF16)
    wq = (Wq.reshape(D, D).astype(np.float32) / np.sqrt(DH)).astype(BF16)
    wk = Wk.reshape(D, D).astype(BF16)
    wv = Wv.reshape(D, D).astype(BF16)
    wo = Wo.reshape(D, D).astype(BF16)

    # per-example side aggregates (sum of x over 16-token groups), f32 sum
    g_all = x.reshape(B, G, TPB, D).sum(2).astype(BF16)

    in_maps = []
    for c in range(N_CORES):
        b, s = c // 4, c % 4
        S0 = s * TOK
        blk0 = S0 // BL
        xkv = np.zeros((KV, D), BF16)
        a0 = S0 - BL
        lo, hi = max(a0, 0), min(a0 + KV, L)
        xkv[lo - a0:hi - a0] = xbf[b, lo:hi]
        # maskT[k, n, c, q]: local-window validity, transposed
        k_ = np.arange(BL)[:, None, None, None]
        n_ = np.arange(NB)[None, :, None, None]
        c_ = np.arange(3)[None, None, :, None]
        q_ = np.arange(BL)[None, None, None, :]
        rel = (c_ * BL + k_) - BL - q_
        kpos = (blk0 + n_ - 1) * BL + c_ * BL + k_
        valid = (np.abs(rel) <= BL - 1) & (kpos >= 0) & (kpos < L)
        in_maps.append({
            "xkv": xkv,
            "g": g_all[b],
            "wq": wq, "wk": wk, "wv": wv, "wo": wo,
            "maskT": valid.astype(BF16),
        })
    return in_maps


_RUNNER = None


def _make_runner(nc):
    """Build the PJRT executable once; returns fn(in_maps) -> per-core outs.

    Mirrors concourse.bass2jax.run_bass_via_pjrt, but caches the jitted
    shard_map callable so repeat kernel() calls skip retrace/recompile.
    """
    import jax
    import numpy as _np
    from jax.sharding import Mesh, PartitionSpec
    from jax.experimental.shard_map import shard_map
    import concourse.mybir as mybir
    from concourse import bass2jax

    bass2jax.install_neuronx_cc_hook()
    partition_name = (nc.partition_id_tensor.name
                      if nc.partition_id_tensor else None)
    in_names, out_names, out_avals = [], [], []
    for alloc in nc.m.functions[0].allocations:
        if not isinstance(alloc, mybir.MemoryLocationSet):
            continue
        name = alloc.memorylocations[0].name
        if alloc.kind == "ExternalInput":
            if name != partition_name:
                in_names.append(name)
        elif alloc.kind == "ExternalOutput":
            out_avals.append(jax.core.ShapedArray(
                tuple(alloc.tensor_shape), mybir.dt.np(alloc.dtype)))
            out_names.append(name)
    n_params = len(in_names)
    all_names = in_names + out_names
    if partition_name is not None:
        all_names.append(partition_name)
    donate = tuple(range(n_params, n_params + len(out_names)))

    def _body(*args):
        operands = list(args)
        if partition_name is not None:
            operands.append(bass2jax.partition_id_tensor())
        return tuple(bass2jax._bass_exec_p.bind(
            *operands, out_avals=tuple(out_avals), in_names=tuple(all_names),
            out_names=tuple(out_names), lowering_input_output_aliases=(),
            sim_require_finite=True, sim_require_nnan=True, nc=nc))

    devices = jax.devices()[:N_CORES]
    mesh = Mesh(_np.asarray(devices), ("core",))
    specs = (PartitionSpec("core"),) * (n_params + len(out_names))
    sharded = jax.jit(
        shard_map(_body, mesh=mesh, in_specs=specs,
                  out_specs=(PartitionSpec("core"),) * len(out_names),
                  check_rep=False),
        donate_argnums=donate, keep_unused=True)

    def run(in_maps):
        concat_in = [
            _np.concatenate([_np.asarray(in_maps[c][k]) for c in range(N_CORES)],
                            axis=0)
            for k in in_names
        ]
        concat_zeros = [_np.zeros((N_CORES * a.shape[0], *a.shape[1:]), a.dtype)
                        for a in out_avals]
        outs = sharded(*concat_in, *concat_zeros)
        return [
            {k: _np.asarray(outs[i]).reshape(N_CORES, *out_avals[i].shape)[c]
             for i, k in enumerate(out_names)}
            for c in range(N_CORES)
        ]

    return run


def kernel(x, Wq, Wk, Wv, Wo):
    global _PROG, _RUNNER
    if _RUNNER is None:
        _PROG = _build_program()
        _RUNNER = _make_runner(_PROG)
    in_maps = _host_inputs(np.asarray(x, np.float32), np.asarray(Wq, np.float32),
                           np.asarray(Wk, np.float32), np.asarray(Wv, np.float32),
                           np.asarray(Wo, np.float32))
    results = _RUNNER(in_maps)
    out = np.empty((B, L, D), np.float32)
    for c in range(N_CORES):
        b, s = c // 4, c % 4
        out[b, s * TOK:(s + 1) * TOK] = results[c]["out"]
    return out


# revision 33
# speedup vs baseline: 15551.4480x; 1.0251x over previous
"""ETC transient-global self-attention on 8 TRN2 NeuronCores.

Sharding: sequence-parallel. Core c handles example b = c//4, token rows
[1024*(c%4), 1024*(c%4+1)). Each core computes its q/k/v projections (k/v
with a 1-block halo), the per-example global (side) aggregates from the
full example, local+global attention, and the output projection for its
token rows. No cross-core communication; the host stacks the 8 row-slices.

Shapes (hardcoded from the problem spec):
  x  [2, 4096, 1024], Wq/Wk/Wv [1024, 16, 64], Wo [16, 64, 1024]
  block_len 128, 32 blocks, TOKENS_PER_BLOCK 16 -> G = 256 side tokens.

On-device layout notes:
  - everything runs in bf16 on the PE (f32 accumulate in PSUM).
  - attention logits are computed TRANSPOSED ([keys, q]) so that exp() is
    orientation-agnostic and PV contracts keys on the partition dim with no
    probs transpose. Softmax denominators come from a ones-column appended
    to v (PSUM row 64); the reference's extra-logit softmax needs no
    max-subtraction since logits are O(1) and masked entries multiply to 0.
"""

import numpy as np
import ml_dtypes

B, L, D, H, DH = 2, 4096, 1024, 16, 64
BL = 128                 # block length
NBLK = 32                # total blocks
G = 256                  # side (global) tokens
TPB = 16                 # tokens per side block
N_CORES = 8
NB = 8                   # blocks per core
TOK = NB * BL            # 1024 q tokens per core
KV = TOK + 2 * BL        # 1280 kv tokens (1-block halo each side)
BF16 = ml_dtypes.bfloat16

_PROG = None


def _build_program():
    import concourse.bass as bass
    import concourse.mybir as mybir
    import concourse.tile as tile
    from concourse import bacc
    from concourse.masks import make_identity

    dt = mybir.dt
    f32, bf16 = dt.float32, dt.bfloat16

    nc = bacc.Bacc("TRN2", target_bir_lowering=False, debug=False,
                   num_devices=N_CORES)

    xkv_d = nc.declare_dram_parameter("xkv", [KV, D], bf16, isOutput=False)
    g_d = nc.declare_dram_parameter("g", [G, D], bf16, isOutput=False)
    wq_d = nc.declare_dram_parameter("wq", [D, D], bf16, isOutput=False)
    wk_d = nc.declare_dram_parameter("wk", [D, D], bf16, isOutput=False)
    wv_d = nc.declare_dram_parameter("wv", [D, D], bf16, isOutput=False)
    wo_d = nc.declare_dram_parameter("wo", [D, D], bf16, isOutput=False)
    msk_d = nc.declare_dram_parameter("maskT", [BL, NB, 3, BL], bf16,
                                      isOutput=False)
    out_d = nc.declare_dram_parameter("out", [TOK, D], f32, isOutput=True)

    Exp = mybir.ActivationFunctionType.Exp
    Cpy = mybir.ActivationFunctionType.Copy

    with tile.TileContext(nc) as tc:
        with (
            tc.tile_pool(name="per", bufs=1) as per,
            tc.tile_pool(name="strm", bufs=3) as strm,
            tc.tile_pool(name="att", bufs=2) as att,
            tc.tile_pool(name="osb", bufs=2) as osb,
        ):
            # ---- persistent SBUF tiles ----
            wk_sb = per.tile([128, 8, D], bf16, tag="wk")
            wv_sb = per.tile([128, 8, D], bf16, tag="wv")
            wo_sb = per.tile([128, 8, D], bf16, tag="wo")
            wq_sb = per.tile([128, 8, D], bf16, tag="wq_yT")   # dies -> yTf
            msk_sb = per.tile([128, NB, 3, BL], bf16, tag="msk")
            ident = per.tile([128, 128], bf16, tag="ident")
            ones64 = per.tile([128, 64], bf16, tag="ones64")
            gnat = per.tile([128, 2, D], bf16, tag="gnat")
            gT = per.tile([128, 8, G], bf16, tag="gT")
            skT = per.tile([128, 8, G], bf16, tag="skT")
            svaug = per.tile([128, 2, H, DH + 1], bf16, tag="svaug")
            qT = per.tile([128, 8, TOK], bf16, tag="qT")
            kT = per.tile([128, 8, KV], bf16, tag="kT")
            vaug = per.tile([128, 10, H, DH + 1], bf16, tag="vaug")
            xT = per.tile([128, 8, KV], bf16, tag="xT_st")     # dies -> stage_o

            # DMA issue order matters for ramp-up: wk/wq land first so the
            # first kT accumulation group can chase the xT transpose chunks
            # as they arrive instead of waiting for the whole queue.
            nc.sync.dma_start(out=wq_sb,
                              in_=wq_d.ap().rearrange("(c p) d -> p c d", p=128))
            # xT built directly by 2-byte transpose DMAs (one per D-chunk)
            for dc in range(8):
                nc.sync.dma_start(out=xT[:, dc, :],
                                  in_=xkv_d[:, dc * 128:(dc + 1) * 128],
                                  transpose=True)
            nc.sync.dma_start(out=wv_sb,
                              in_=wv_d.ap().rearrange("(c p) d -> p c d", p=128))
            nc.sync.dma_start(out=gnat,
                              in_=g_d.ap().rearrange("(t p) d -> p t d", p=128))
            nc.sync.dma_start(out=msk_sb, in_=msk_d.ap())
            nc.sync.dma_start(out=wk_sb,
                              in_=wk_d.ap().rearrange("(c p) d -> p c d", p=128))
            nc.sync.dma_start(out=wo_sb,
                              in_=wo_d.ap().rearrange("(c p) d -> p c d", p=128))
            make_identity(nc, ident)
            nc.vector.memset(ones64, 1.0)

            with tc.tile_pool(name="pst", bufs=2, space="PSUM") as pst:
                # ---- build gT (xT comes straight from transpose DMA) ----
                for gt_i in range(2):
                    for dc in range(8):
                        pt = pst.tile([128, 128], bf16, tag="tp")
                        nc.tensor.transpose(
                            pt, gnat[:, gt_i, dc * 128:(dc + 1) * 128], ident)
                        nc.scalar.copy(gT[:, dc, gt_i * 128:(gt_i + 1) * 128], pt)

            # Projections (except kT) run first; the kT projection for each
            # head pair is interleaved with that pair's attention so the
            # PE-bound projection work overlaps the ACT-bound exp window.
            with tc.tile_pool(name="pspa", bufs=2, space="PSUM") as psp:
                # qT (q tokens = xT kv-rows 128..1152), Wq pre-scaled by 1/8.
                # Emitted first so wq dies before yTf reuses its SBUF slot.
                for oc in range(8):
                    for tch in range(2):
                        ts_ = 128 + tch * 512
                        pp = psp.tile([128, 512], f32, tag="pj")
                        for dc in range(8):
                            nc.tensor.matmul(
                                pp,
                                wq_sb[:, dc, oc * 128:(oc + 1) * 128],
                                xT[:, dc, ts_:ts_ + 512],
                                start=(dc == 0), stop=(dc == 7))
                        nc.vector.tensor_copy(
                            qT[:, oc, tch * 512:(tch + 1) * 512], pp)
                # v natural (augmented with ones column per head)
                for t in range(10):
                    for j in range(2):
                        pp = psp.tile([128, 512], f32, tag="pj")
                        for dc in range(8):
                            nc.tensor.matmul(
                                pp,
                                xT[:, dc, t * 128:(t + 1) * 128],
                                wv_sb[:, dc, 512 * j:512 * (j + 1)],
                                start=(dc == 0), stop=(dc == 7))
                        nc.scalar.copy(
                            vaug[:, t, 8 * j:8 * (j + 1), 0:DH],
                            pp.rearrange("p (h d) -> p h d", h=8))
                    nc.vector.memset(vaug[:, t, :, DH:DH + 1], 1.0)
                # side kT
                for oc in range(8):
                    pp = psp.tile([128, 512], f32, tag="pj")
                    for dc in range(8):
                        nc.tensor.matmul(
                            pp[:, :G],
                            wk_sb[:, dc, oc * 128:(oc + 1) * 128],
                            gT[:, dc, :],
                            start=(dc == 0), stop=(dc == 7))
                    nc.vector.tensor_copy(skT[:, oc, :], pp[:, :G])
                # side v (augmented)
                for gt_i in range(2):
                    for j in range(2):
                        pp = psp.tile([128, 512], f32, tag="pj")
                        for dc in range(8):
                            nc.tensor.matmul(
                                pp,
                                gT[:, dc, gt_i * 128:(gt_i + 1) * 128],
                                wv_sb[:, dc, 512 * j:512 * (j + 1)],
                                start=(dc == 0), stop=(dc == 7))
                        nc.scalar.copy(
                            svaug[:, gt_i, 8 * j:8 * (j + 1), 0:DH],
                            pp.rearrange("p (h d) -> p h d", h=8))
                    nc.vector.memset(svaug[:, gt_i, :, DH:DH + 1], 1.0)

            # ---- kT projection + attention, interleaved per head pair
            with tc.tile_pool(name="psp", bufs=1, space="PSUM") as psp, \
                 tc.tile_pool(name="plg", bufs=2, space="PSUM") as plg, \
                 tc.tile_pool(name="psg2", bufs=1, space="PSUM") as psg2, \
                 tc.tile_pool(name="pyt", bufs=2, space="PSUM") as pyt, \
                 tc.tile_pool(name="pbc", bufs=1, space="PSUM") as pbc:
                yTf = per.tile([128, 8, TOK], bf16, tag="wq_yT")

                def attn_head(h, stg):
                    oc, r0 = h // 2, 64 * (h % 2)
                    for nh in range(2):          # half = 4 blocks = 512 q
                        q4 = qT[r0:r0 + 64, oc, nh * 512:(nh + 1) * 512]
                        # side QK batched over the 4 blocks (N=512), one exp
                        sg = psg2.tile([128, 2, 512], f32, tag="sg",
                                       name=f"sg{h}_{nh}")
                        us = att.tile([128, 2, 512], bf16, tag="us", bufs=3,
                                      name=f"us{h}_{nh}")
                        for g in range(2):
                            nc.tensor.matmul(
                                sg[:, g, :],
                                skT[r0:r0 + 64, oc, g * 128:(g + 1) * 128],
                                q4, start=True, stop=True)
                        nc.scalar.activation(us, sg, Exp)
                        # local QK + exp + mask per block
                        uls = []
                        for i in range(4):
                            n = nh * 4 + i
                            qs = qT[r0:r0 + 64, oc, n * 128:(n + 1) * 128]
                            lg = plg.tile([128, 3, 128], f32, tag="lg",
                                          name=f"lg{h}_{n}")
                            for c in range(3):
                                nc.tensor.matmul(
                                    lg[:, c, :],
                                    kT[r0:r0 + 64, oc,
                                       (n + c) * 128:(n + c + 1) * 128],
                                    qs, start=True, stop=True)
                            ul = att.tile([128, 3, 128], bf16, tag="ul", bufs=8,
                                          name=f"ul{h}_{n}")
                            nc.scalar.activation(ul, lg, Exp)
                            nc.vector.tensor_mul(ul, ul, msk_sb[:, n, :, :])
                            uls.append(ul)
                        # PV for 4 blocks into one [65, 512] psum
                        yt = pyt.tile([65, 512], f32, tag="yt",
                                      name=f"yt{h}_{nh}")
                        for i in range(4):
                            n = nh * 4 + i
                            for c in range(3):
                                # start=True clears this whole PSUM bank, so
                                # only the very first matmul of the group may
                                # set it; per-element has_written handles the
                                # first write of each column slice.
                                nc.tensor.matmul(
                                    yt[:, i * 128:(i + 1) * 128],
                                    vaug[:, n + c, h, :], uls[i][:, c, :],
                                    start=(i == 0 and c == 0), stop=False)
                        for g in range(2):
                            nc.tensor.matmul(
                                yt, svaug[:, g, h, :], us[:, g, :],
                                start=False, stop=(g == 1))
                        # normalize: bcast (denom+1), lane-parallel recip, mul
                        rc = att.tile([128, 512], bf16, tag="rc", bufs=2,
                                      name=f"rc{h}_{nh}")
                        nc.vector.tensor_scalar_add(rc[64:65, :], yt[64:65, :],
                                                    1.0)
                        bc = pbc.tile([64, 512], f32, tag="bc",
                                      name=f"bc{h}_{nh}")
                        nc.tensor.matmul(bc, ones64[64:65, :], rc[64:65, :],
                                         start=True, stop=True)
                        rcb = att.tile([64, 512], bf16, tag="rcb", bufs=2,
                                       name=f"rcb{h}_{nh}")
                        with nc.allow_low_precision(reason="bf16 softmax recip"):
                            nc.vector.reciprocal(rcb, bc)
                        dst = (yTf[0:64, oc, nh * 512:(nh + 1) * 512]
                               if h % 2 == 0 else
                               stg[:, nh * 512:(nh + 1) * 512])
                        nc.vector.tensor_mul(dst, yt[0:64, :], rcb)

                for oc in range(8):
                    # kT projection for this head pair
                    for ts_, te in ((0, 512), (512, 1024), (1024, 1280)):
                        pp = psp.tile([128, 512], f32, tag="pj",
                                      name=f"ppk{oc}_{ts_}")
                        for dc in range(8):
                            nc.tensor.matmul(
                                pp[:, :te - ts_],
                                wk_sb[:, dc, oc * 128:(oc + 1) * 128],
                                xT[:, dc, ts_:te],
                                start=(dc == 0), stop=(dc == 7))
                        nc.vector.tensor_copy(kT[:, oc, ts_:te],
                                              pp[:, :te - ts_])
                    attn_head(2 * oc, None)
                    stg = att.tile([64, TOK], bf16, tag="stg", bufs=2,
                                   name=f"stg{oc}")
                    attn_head(2 * oc + 1, stg)
                    # shift this pair's odd head up to partitions 64..127
                    nc.sync.dma_start(out=yTf[64:128, oc, :], in_=stg)

            # ---- phase F: output projection ----
            with tc.tile_pool(name="pso", bufs=2, space="PSUM") as pso:
                for tt in range(8):
                    ot = osb.tile([128, D], f32, tag="ot")
                    for j in range(2):
                        pp = pso.tile([128, 512], f32, tag="po")
                        for oc in range(8):
                            nc.tensor.matmul(
                                pp,
                                yTf[:, oc, tt * 128:(tt + 1) * 128],
                                wo_sb[:, oc, 512 * j:512 * (j + 1)],
                                start=(oc == 0), stop=(oc == 7))
                        nc.vector.tensor_copy(ot[:, 512 * j:512 * (j + 1)], pp)
                    nc.sync.dma_start(out=out_d[tt * 128:(tt + 1) * 128, :],
                                      in_=ot)

    nc.compile()
    return nc


def _host_inputs(x, Wq, Wk, Wv, Wo):
    """Build the 8 per-core input maps (all numpy, bf16 where device expects)."""
    xbf = x.astype(BF16)
    wq = (Wq.reshape(D, D).astype(np.float32) / np.sqrt(DH)).astype(BF16)
    wk = Wk.reshape(D, D).astype(BF16)
    wv = Wv.reshape(D, D).astype(BF16)
    wo = Wo.reshape(D, D).astype(BF16)

    # per-example side aggregates (sum of x over 16-token groups), f32 sum
    g_all = x.reshape(B, G, TPB, D).sum(2).astype(BF16)

    in_maps = []
    for c in range(N_CORES):
        b, s = c // 4, c % 4
        S0 = s * TOK
        blk0 = S0 // BL
        xkv = np.zeros((KV, D), BF16)
        a0 = S0 - BL
        lo, hi = max(a0, 0), min(a0 + KV, L)
        xkv[lo - a0:hi - a0] = xbf[b, lo:hi]
        # maskT[k, n, c, q]: local-window validity, transposed
        k_ = np.arange(BL)[:, None, None, None]
        n_ = np.arange(NB)[None, :, None, None]
        c_ = np.arange(3)[None, None, :, None]
        q_ = np.arange(BL)[None, None, None, :]
        rel = (c_ * BL + k_) - BL - q_
        kpos = (blk0 + n_ - 1) * BL + c_ * BL + k_
        valid = (np.abs(rel) <= BL - 1) & (kpos >= 0) & (kpos < L)
        in_maps.append({
            "xkv": xkv,
            "g": g_all[b],
            "wq": wq, "wk": wk, "wv": wv, "wo": wo,
            "maskT": valid.astype(BF16),
        })
    return in_maps


_RUNNER = None


def _make_runner(nc):
    """Build the PJRT executable once; returns fn(in_maps) -> per-core outs.

    Mirrors concourse.bass2jax.run_bass_via_pjrt, but caches the jitted
    shard_map callable so repeat kernel() calls skip retrace/recompile.
    """
    import jax
    import numpy as _np
    from jax.sharding import Mesh, PartitionSpec
    from jax.experimental.shard_map import shard_map
    import concourse.mybir as mybir
    from concourse import bass2jax

    bass2jax.install_neuronx_cc_hook()
    partition_name = (nc.partition_id_tensor.name
                      if nc.partition_id_tensor else None)
    in_names, out_names, out_avals = [], [], []
    for alloc in nc.m.functions[0].allocations:
        if not isinstance(alloc, mybir.MemoryLocationSet):
            continue
        name = alloc.memorylocations[0].name
        if alloc.kind == "ExternalInput":
            if name != partition_name:
                in_names.append(name)
        elif alloc.kind == "ExternalOutput":
            out_avals.append(jax.core.ShapedArray(
                tuple(alloc.tensor_shape), mybir.dt.np(alloc.dtype)))
            out_names.append(name)
    n_params = len(in_names)
    all_names = in_names + out_names
    if partition_name is not None:
        all_names.append(partition_name)
    donate = tuple(range(n_params, n_params + len(out_names)))

    def _body(*args):
        operands = list(args)
        if partition_name is not None:
            operands.append(bass2jax.partition_id_tensor())
        return tuple(bass2jax._bass_exec_p.bind(
            *operands, out_avals=tuple(out_avals), in_names=tuple(all_names),
            out_names=tuple(out_names), lowering_input_output_aliases=(),
            sim_require_finite=True, sim_require_nnan=True, nc=nc))

    devices = jax.devices()[:N_CORES]
    mesh = Mesh(_np.asarray(devices), ("core",))
    specs = (PartitionSpec("core"),) * (n_params + len(out_names))
    sharded = jax.jit(
        shard_map(_body, mesh=mesh, in_specs=specs,
                  out_specs=(PartitionSpec("core"),) * len(out_names),
                  check_rep=False),
        donate_argnums=donate, keep_unused=True)

    def run(in_maps):
        concat_in = [
            _np.concatenate([_np.asarray(in_maps[c][k]) for c in range(N_CORES)],
                            axis=0)
            for k in in_names
        ]
        concat_zeros = [_np.zeros((N_CORES * a.shape[0], *a.shape[1:]), a.dtype)
                        for a in out_avals]
        outs = sharded(*concat_in, *concat_zeros)
        return [
            {k: _np.asarray(outs[i]).reshape(N_CORES, *out_avals[i].shape)[c]
             for i, k in enumerate(out_names)}
            for c in range(N_CORES)
        ]

    return run


def kernel(x, Wq, Wk, Wv, Wo):
    global _PROG, _RUNNER
    if _RUNNER is None:
        _PROG = _build_program()
        _RUNNER = _make_runner(_PROG)
    in_maps = _host_inputs(np.asarray(x, np.float32), np.asarray(Wq, np.float32),
                           np.asarray(Wk, np.float32), np.asarray(Wv, np.float32),
                           np.asarray(Wo, np.float32))
    results = _RUNNER(in_maps)
    out = np.empty((B, L, D), np.float32)
    for c in range(N_CORES):
        b, s = c // 4, c % 4
        out[b, s * TOK:(s + 1) * TOK] = results[c]["out"]
    return out
